# revision 9
# baseline (speedup 1.0000x reference)
"""Trainium2 Bass kernel for nn_AdaptiveSampler (sparse grid_sample attention).

Strategy (data-parallel over batch, 8 cores x 4 batch items each):
  - Host: features reshaped channels-last [B*H*W, C] in bf16 so every
    spatial cell is one contiguous 2KB row -> indirect row gathers.
  - Device per core:
      phase A: keypoint -> bilinear corner cells/weights (DVE f32 math)
      seed    = dma_gather(4 corners x 512 keypoints) -> weighted reduce
      MLPs    = PE matmuls (offsets + attention logits), softmax on DVE/ACT
      phase B: 16 corner cells/weights per keypoint (4 pts x 4 corners)
      fused   = per-batch dma_gather(2048 rows) * broadcast weights,
                segment-reduce over 16, PE-transpose to [j, c], DMA out.
All computation (gathers, MLPs, softmax, bilinear) happens on-device; the
host only reorders input layout and concatenates per-core outputs.
"""

import os
import sys
from contextlib import ExitStack

import numpy as np

sys.path.insert(0, "/opt/trn_rl_repo")

import ml_dtypes

import concourse.bass as bass
import concourse.tile as tile
from concourse import bacc, mybir

F32 = mybir.dt.float32
BF16 = mybir.dt.bfloat16
I16 = mybir.dt.int16

ALU = mybir.AluOpType
ACT = mybir.ActivationFunctionType
AX = mybir.AxisListType

B = 4          # batch items per core
C = 1024       # channels
H = W = 64
HW = H * W     # 4096 cells per batch item
J = 128        # keypoints
NP = 4         # sample points per keypoint
Q = C // 128   # 8 channel chunks
NIDX = J * 16  # 2048 indices per gather (seed: J*B*4 ; main: per-b J*16)
TWO23 = float(2 ** 23)


def _floor(nc, pool, src, shape):
    """floor(src) on DVE via round-to-nearest + correction. Returns tile."""
    rnd = pool.tile(list(shape), F32, tag="floor_rnd")
    nc.vector.tensor_scalar(rnd[:], src, TWO23, TWO23, ALU.add, ALU.subtract)
    flo = pool.tile(list(shape), F32, tag="floor_out")
    # flo = (src < rnd) ? 1 : 0 ; then flo = rnd - flo
    nc.vector.tensor_tensor(flo[:], src, rnd[:], ALU.is_lt)
    nc.vector.tensor_tensor(flo[:], rnd[:], flo[:], ALU.subtract)
    return flo


def build_nc():
    nc = bacc.Bacc()

    feat = nc.declare_dram_parameter("feat", [B * HW, C], BF16, isOutput=False)
    kp = nc.declare_dram_parameter("kp", [J, 2 * B], F32, isOutput=False)
    w1o = nc.declare_dram_parameter("w1o", [128, Q, 128], BF16, isOutput=False)
    w1a = nc.declare_dram_parameter("w1a", [128, Q, 128], BF16, isOutput=False)
    w2o = nc.declare_dram_parameter("w2o", [128, 8], BF16, isOutput=False)
    w2a = nc.declare_dram_parameter("w2a", [128, 4], BF16, isOutput=False)
    b1o = nc.declare_dram_parameter("b1o", [128, 1], F32, isOutput=False)
    b1a = nc.declare_dram_parameter("b1a", [128, 1], F32, isOutput=False)
    b2o = nc.declare_dram_parameter("b2o", [8, 1], F32, isOutput=False)
    b2a = nc.declare_dram_parameter("b2a", [4, 1], F32, isOutput=False)
    bbase = nc.declare_dram_parameter("bbase", [128, B], F32, isOutput=False)
    ident = nc.declare_dram_parameter("ident", [128, 128], F32, isOutput=False)
    out = nc.declare_dram_parameter("out", [B * J, C], F32, isOutput=True)

    # DRAM scratch for flattening per-column weights before partition bcast
    wscr = nc.dram_tensor("wscr", [B + 1, J * 16], BF16)

    with ExitStack() as ctx:
        tc = ctx.enter_context(tile.TileContext(nc))
        cons = ctx.enter_context(tc.tile_pool(name="cons", bufs=1))
        a = ctx.enter_context(tc.tile_pool(name="phaseA", bufs=1))
        gp = ctx.enter_context(tc.tile_pool(name="gather", bufs=2))
        wp = ctx.enter_context(tc.tile_pool(name="wbc", bufs=2))
        op = ctx.enter_context(tc.tile_pool(name="outT", bufs=2))
        ip = ctx.enter_context(tc.tile_pool(name="idxw", bufs=2))
        ps = ctx.enter_context(tc.tile_pool(name="psT", bufs=3, space="PSUM"))
        pmm = ctx.enter_context(tc.tile_pool(name="psMM", bufs=3, space="PSUM"))

        # ---------------- constants ----------------
        kp_sb = cons.tile([J, B, 2], F32)
        nc.sync.dma_start(out=kp_sb[:], in_=kp[:].rearrange("j (b t) -> j b t", t=2))
        w1o_sb = cons.tile([128, Q, 128], BF16)
        nc.sync.dma_start(out=w1o_sb[:], in_=w1o[:])
        w1a_sb = cons.tile([128, Q, 128], BF16)
        nc.sync.dma_start(out=w1a_sb[:], in_=w1a[:])
        w2o_sb = cons.tile([128, 8], BF16)
        nc.sync.dma_start(out=w2o_sb[:], in_=w2o[:])
        w2a_sb = cons.tile([128, 4], BF16)
        nc.sync.dma_start(out=w2a_sb[:], in_=w2a[:])
        b1o_sb = cons.tile([128, 1], F32)
        nc.sync.dma_start(out=b1o_sb[:], in_=b1o[:])
        b1a_sb = cons.tile([128, 1], F32)
        nc.sync.dma_start(out=b1a_sb[:], in_=b1a[:])
        b2o_sb = cons.tile([8, 1], F32)
        nc.sync.dma_start(out=b2o_sb[:], in_=b2o[:])
        b2a_sb = cons.tile([4, 1], F32)
        nc.sync.dma_start(out=b2a_sb[:], in_=b2a[:])
        bbase_sb = cons.tile([128, B], F32)
        nc.sync.dma_start(out=bbase_sb[:], in_=bbase[:])
        id_sb = cons.tile([128, 128], F32)
        nc.sync.dma_start(out=id_sb[:], in_=ident[:])

        # ---------------- phase A: seed corners ----------------
        # pixel coords: ix = (x+1)*31.5, [J, B]
        ix = a.tile([J, B], F32)
        nc.vector.tensor_scalar(ix[:], kp_sb[:, :, 0], 31.5, 31.5, ALU.mult, ALU.add)
        iy = a.tile([J, B], F32)
        nc.vector.tensor_scalar(iy[:], kp_sb[:, :, 1], 31.5, 31.5, ALU.mult, ALU.add)

        x0 = _floor(nc, a, ix[:], (J, B))
        y0 = _floor(nc, a, iy[:], (J, B))

        # xp [J,B,2] = (x0, x0+1); wxp [J,B,2] = (1-fx, fx); same for y
        def pair_and_weights(base, i_coord, tagp):
            p = a.tile([J, B, 2], F32, tag=f"{tagp}_p")
            wgt = a.tile([J, B, 2], F32, tag=f"{tagp}_w")
            nc.vector.tensor_copy(p[:, :, 0], base[:])
            nc.vector.tensor_scalar_add(p[:, :, 1], base[:], 1.0)
            # fx = i - x0 -> wgt1 ; wgt0 = 1 - fx
            nc.vector.tensor_tensor(wgt[:, :, 1], i_coord, base[:], ALU.subtract)
            nc.vector.tensor_scalar(
                wgt[:, :, 0], wgt[:, :, 1], -1.0, 1.0, ALU.mult, ALU.add
            )
            return p, wgt

        xp, wxp = pair_and_weights(x0, ix[:], "x")
        yp, wyp = pair_and_weights(y0, iy[:], "y")

        # seed cell idx [J, B, 2cy, 2cx] = bbase + yp*64 + xp
        idx4 = a.tile([J, B, 2, 2], F32)
        t1 = a.tile([J, B, 2], F32)
        nc.vector.tensor_scalar_mul(t1[:], yp[:], 64.0)
        nc.vector.tensor_tensor(
            idx4[:],
            t1[:].unsqueeze(3).to_broadcast((J, B, 2, 2)),
            xp[:].unsqueeze(2).to_broadcast((J, B, 2, 2)),
            ALU.add,
        )
        nc.vector.tensor_tensor(
            idx4[:],
            idx4[:],
            bbase_sb[:].unsqueeze(2).unsqueeze(3).to_broadcast((J, B, 2, 2)),
            ALU.add,
        )
        w4 = a.tile([J, B, 2, 2], F32)
        nc.vector.tensor_tensor(
            w4[:],
            wyp[:].unsqueeze(3).to_broadcast((J, B, 2, 2)),
            wxp[:].unsqueeze(2).to_broadcast((J, B, 2, 2)),
            ALU.mult,
        )

        def wrap_idx(idx_flat_ap):
            """[J,16] f32 cell ids -> wrapped+replicated [128, J] int16 tile."""
            rep = ip.tile([J, 8, 16], F32, tag="idxrep")
            for g in range(8):
                nc.vector.tensor_copy(rep[:, g, :], idx_flat_ap)
            psT = ps.tile([128, J], F32, tag="tp")
            nc.tensor.transpose(
                psT[:], rep[:].rearrange("j g c -> j (g c)"), id_sb[:, :J]
            )
            idxw = ip.tile([128, J], I16, tag="idxw")
            nc.vector.tensor_copy(idxw[:], psT[:])
            return idxw

        def bcast_weights(w_flat_ap, slot):
            """[J,16] f32 col-weights -> [128, NIDX] bf16 via DRAM bounce."""
            wb16 = a.tile([J, 16], BF16, tag=f"wb16_{slot}")
            nc.vector.tensor_copy(wb16[:], w_flat_ap)
            nc.sync.dma_start(
                out=wscr[slot].rearrange("(j c) -> j c", c=16), in_=wb16[:]
            )
            wbc = wp.tile([128, NIDX], BF16, tag="wbc")
            nc.sync.dma_start(
                out=wbc[:],
                in_=wscr[slot].unsqueeze(0).to_broadcast((128, NIDX)),
            )
            return wbc

        idxw_seed = wrap_idx(idx4[:].rearrange("j b cy cx -> j (b cy cx)"))
        wbc_seed = bcast_weights(w4[:].rearrange("j b cy cx -> j (b cy cx)"), B)

        # ---------------- seed gather + combine ----------------
        HN = NIDX // 8   # 256 indices per gather chunk (pipeline in 512-desc ring)
        seed = a.tile([128, Q, J * B], BF16)
        for h in range(8):
            seedg = gp.tile([128, Q, HN], BF16, tag="seedg")
            nc.gpsimd.dma_gather(
                seedg[:],
                feat[:],
                idxw_seed[:, 16 * h : 16 * h + 16],
                num_idxs=HN,
                num_idxs_reg=HN,
                elem_size=C,
                transpose=True,
            )
            with nc.allow_low_precision("bf16 grid-sample compute"):
                nc.vector.tensor_tensor(
                    seedg[:],
                    seedg[:],
                    wbc_seed[:, HN * h : HN * (h + 1)]
                    .unsqueeze(1)
                    .to_broadcast((128, Q, HN)),
                    ALU.mult,
                )
                nc.vector.tensor_reduce(
                    seed[:, :, 64 * h : 64 * (h + 1)],
                    seedg[:].rearrange("p q (jb c) -> p q jb c", c=4),
                    AX.X,
                    ALU.add,
                )

        # ---------------- MLPs ----------------
        def mlp_head(w1_sb, b1_sb, name):
            hps = pmm.tile([128, J * B], F32, tag="mm")
            for q in range(Q):
                nc.tensor.matmul(
                    hps[:],
                    w1_sb[:, q, :],
                    seed[:, q, :],
                    start=(q == 0),
                    stop=(q == Q - 1),
                )
            h_sb = a.tile([128, J * B], BF16, tag=f"hsb_{name}")
            nc.scalar.activation(h_sb[:], hps[:], ACT.Relu, bias=b1_sb[:])
            return h_sb

        h_off = mlp_head(w1o_sb, b1o_sb, "off")
        h_att = mlp_head(w1a_sb, b1a_sb, "att")

        ops2 = pmm.tile([8, J * B], F32, tag="mm")
        nc.tensor.matmul(ops2[:], w2o_sb[:], h_off[:], start=True, stop=True)
        off2 = a.tile([8, J * B], F32)
        nc.scalar.activation(off2[:], ops2[:], ACT.Identity, bias=b2o_sb[:])

        aps2 = pmm.tile([4, J * B], F32, tag="mm")
        nc.tensor.matmul(aps2[:], w2a_sb[:], h_att[:], start=True, stop=True)
        att2 = a.tile([4, J * B], F32)
        nc.scalar.activation(att2[:], aps2[:], ACT.Identity, bias=b2a_sb[:])

        # transpose MLP outputs back to [J, B, ch] layout (per-b strided cols)
        offT = a.tile([J, B, 8], F32)
        attT = a.tile([J, B, 4], F32)
        for b in range(B):
            pso = ps.tile([J, 8], F32, tag="tp")
            nc.tensor.transpose(pso[:], off2[:, b::B], id_sb[:8, :8])
            nc.scalar.copy(offT[:, b, :], pso[:])
            psa = ps.tile([J, 4], F32, tag="tp")
            nc.tensor.transpose(psa[:], att2[:, b::B], id_sb[:4, :4])
            nc.scalar.copy(attT[:, b, :], psa[:])

        # ---------------- phase B: per-point corners ----------------
        # px/py [J, B, NP]
        px = a.tile([J, B, NP], F32)
        nc.vector.tensor_tensor(
            px[:],
            ix[:].unsqueeze(2).to_broadcast((J, B, NP)),
            offT[:, :, 0:NP],
            ALU.add,
        )
        py = a.tile([J, B, NP], F32)
        nc.vector.tensor_tensor(
            py[:],
            iy[:].unsqueeze(2).to_broadcast((J, B, NP)),
            offT[:, :, NP : 2 * NP],
            ALU.add,
        )

        def corner2(pc, tagp):
            """coords pc [J,B,NP] -> clamped pair [J,B,NP,2], masked wpair."""
            c0 = _floor(nc, a, pc[:], (J, B, NP))
            pair = a.tile([J, B, NP, 2], F32, tag=f"{tagp}_pair")
            wgt = a.tile([J, B, NP, 2], F32, tag=f"{tagp}_wgt")
            vmask = a.tile([J, B, NP], F32, tag=f"{tagp}_v")
            # frac -> w1; w0 = 1-frac
            nc.vector.tensor_tensor(wgt[:, :, :, 1], pc[:], c0[:], ALU.subtract)
            nc.vector.tensor_scalar(
                wgt[:, :, :, 0], wgt[:, :, :, 1], -1.0, 1.0, ALU.mult, ALU.add
            )
            # validity: c0 >= 0 ; c1 = c0+1 <= 63
            nc.vector.tensor_scalar(vmask[:], c0[:], 0.0, None, ALU.is_ge)
            nc.vector.tensor_tensor(wgt[:, :, :, 0], wgt[:, :, :, 0], vmask[:], ALU.mult)
            nc.vector.tensor_scalar(vmask[:], c0[:], 62.5, None, ALU.is_lt)
            nc.vector.tensor_tensor(wgt[:, :, :, 1], wgt[:, :, :, 1], vmask[:], ALU.mult)
            # clamped coords
            nc.vector.tensor_scalar_max(pair[:, :, :, 0], c0[:], 0.0)
            nc.vector.tensor_scalar(
                pair[:, :, :, 1], c0[:], 1.0, 63.0, ALU.add, ALU.min
            )
            return pair, wgt

        xpair, wxg = corner2(px, "px")
        ypair, wyg = corner2(py, "py")

        # softmax over NP  [J, B, NP]
        amax = a.tile([J, B, 1], F32)
        nc.vector.tensor_reduce(amax[:], attT[:], AX.X, ALU.max)
        ae = a.tile([J, B, NP], F32)
        nc.vector.tensor_tensor(
            ae[:], attT[:], amax[:].to_broadcast((J, B, NP)), ALU.subtract
        )
        nc.scalar.activation(ae[:], ae[:], ACT.Exp)
        asum = a.tile([J, B, 1], F32)
        nc.vector.tensor_reduce(asum[:], ae[:], AX.X, ALU.add)
        nc.vector.reciprocal(asum[:], asum[:])
        attw = a.tile([J, B, NP], F32)
        nc.vector.tensor_tensor(
            attw[:], ae[:], asum[:].to_broadcast((J, B, NP)), ALU.mult
        )

        # combined weights [J, B, NP, 2cy, 2cx] and cells
        s1 = a.tile([J, B, NP, 2], F32)
        nc.vector.tensor_tensor(
            s1[:], attw[:].unsqueeze(3).to_broadcast((J, B, NP, 2)), wyg[:], ALU.mult
        )
        BN = B * NP
        w16 = a.tile([J, B, NP, 2, 2], F32)
        nc.vector.tensor_tensor(
            w16[:].rearrange("j b n cy cx -> j (b n) cy cx"),
            s1[:].rearrange("j b n cy -> j (b n) cy")
            .unsqueeze(3)
            .to_broadcast((J, BN, 2, 2)),
            wxg[:].rearrange("j b n cx -> j (b n) cx")
            .unsqueeze(2)
            .to_broadcast((J, BN, 2, 2)),
            ALU.mult,
        )
        idx16 = a.tile([J, B, NP, 2, 2], F32)
        t2 = a.tile([J, B, NP, 2], F32)
        nc.vector.tensor_scalar_mul(t2[:], ypair[:], 64.0)
        nc.vector.tensor_tensor(
            idx16[:].rearrange("j b n cy cx -> j (b n) cy cx"),
            t2[:].rearrange("j b n cy -> j (b n) cy")
            .unsqueeze(3)
            .to_broadcast((J, BN, 2, 2)),
            xpair[:].rearrange("j b n cx -> j (b n) cx")
            .unsqueeze(2)
            .to_broadcast((J, BN, 2, 2)),
            ALU.add,
        )
        nc.vector.tensor_tensor(
            idx16[:].rearrange("j b n cy cx -> j b (n cy cx)"),
            idx16[:].rearrange("j b n cy cx -> j b (n cy cx)"),
            bbase_sb[:].unsqueeze(2).to_broadcast((J, B, 4 * NP)),
            ALU.add,
        )

        # ---------------- phase B: gather + fuse per batch item ----------------
        for b in range(B):
            idxw_b = wrap_idx(
                idx16[:, b, :, :, :].rearrange("j n cy cx -> j (n cy cx)")
            )
            wbc_b = bcast_weights(
                w16[:, b, :, :, :].rearrange("j n cy cx -> j (n cy cx)"), b
            )
            fused = gp.tile([128, Q, J], F32, tag="fused")
            for h in range(8):
                mg = gp.tile([128, Q, HN], BF16, tag="mg")
                nc.gpsimd.dma_gather(
                    mg[:],
                    feat[:],
                    idxw_b[:, 16 * h : 16 * h + 16],
                    num_idxs=HN,
                    num_idxs_reg=HN,
                    elem_size=C,
                    transpose=True,
                )
                with nc.allow_low_precision("bf16 grid-sample compute"):
                    nc.vector.tensor_tensor(
                        mg[:],
                        mg[:],
                        wbc_b[:, HN * h : HN * (h + 1)]
                        .unsqueeze(1)
                        .to_broadcast((128, Q, HN)),
                        ALU.mult,
                    )
                nc.vector.tensor_reduce(
                    fused[:, :, 16 * h : 16 * (h + 1)],
                    mg[:].rearrange("p q (j c) -> p q j c", c=16),
                    AX.X,
                    ALU.add,
                )
            outT = op.tile([J, Q, 128], F32, tag="outT")
            for q in range(Q):
                pst = ps.tile([J, 128], F32, tag="tp")
                nc.tensor.transpose(pst[:], fused[:, q, :], id_sb[:, :J])
                nc.scalar.copy(outT[:, q, :], pst[:])
            nc.sync.dma_start(
                out=out[b * J : (b + 1) * J, :].rearrange(
                    "j (q c) -> j q c", q=Q
                ),
                in_=outT[:],
            )

    nc.finalize()
    return nc


def prepare_in_maps(features, keypoint_coords, w_off1, b_off1, w_off2, b_off2,
                    w_att1, b_att1, w_att2, b_att2, n_cores=8):
    bf = ml_dtypes.bfloat16
    f32 = np.float32

    def w1t(w):  # [128, C] -> [128 k_local, Q, 128 m] bf16
        return np.ascontiguousarray(
            w.T.reshape(Q, 128, 128).transpose(1, 0, 2).astype(bf)
        )

    w1o_h = w1t(np.asarray(w_off1, f32))
    w1a_h = w1t(np.asarray(w_att1, f32))
    w2o_h = np.ascontiguousarray(
        np.concatenate([w_off2[0::2], w_off2[1::2]], 0).T.astype(bf)
    )
    w2a_h = np.ascontiguousarray(np.asarray(w_att2, f32).T.astype(bf))
    b1o_h = np.asarray(b_off1, f32).reshape(128, 1).copy()
    b1a_h = np.asarray(b_att1, f32).reshape(128, 1).copy()
    b2o_h = np.concatenate([b_off2[0::2], b_off2[1::2]]).astype(f32).reshape(8, 1)
    b2a_h = np.asarray(b_att2, f32).reshape(4, 1).copy()
    bbase_h = np.broadcast_to(
        (np.arange(B, dtype=f32) * HW)[None, :], (128, B)
    ).copy()
    ident_h = np.eye(128, dtype=f32)

    in_maps = []
    for m in range(n_cores):
        bs = slice(B * m, B * (m + 1))
        feat_h = np.ascontiguousarray(
            np.asarray(features[bs], f32).transpose(0, 2, 3, 1).reshape(B * HW, C)
        ).astype(bf)
        kp_h = np.ascontiguousarray(
            np.asarray(keypoint_coords[bs], f32).transpose(1, 0, 2).reshape(J, 2 * B)
        )
        in_maps.append({
            "feat": feat_h, "kp": kp_h,
            "w1o": w1o_h, "w1a": w1a_h, "w2o": w2o_h, "w2a": w2a_h,
            "b1o": b1o_h, "b1a": b1a_h, "b2o": b2o_h, "b2a": b2a_h,
            "bbase": bbase_h, "ident": ident_h,
        })
    return in_maps


_NC_CACHE = None


def get_nc():
    global _NC_CACHE
    if _NC_CACHE is None:
        _NC_CACHE = build_nc()
    return _NC_CACHE


def kernel(**inputs):
    from concourse.bass_utils import run_bass_kernel_spmd

    n_cores = 8
    nc = get_nc()
    in_maps = prepare_in_maps(**inputs, n_cores=n_cores)
    res = run_bass_kernel_spmd(
        nc, in_maps, core_ids=list(range(n_cores)),
        trace=bool(int(os.environ.get("KERNEL_TRACE", "0") or 0)),
    )
    kernel.last_results = res
    outs = [r["out"].reshape(B, J, C) for r in res.results]
    return np.concatenate(outs, axis=0).astype(np.float32)


# revision 12
# speedup vs baseline: 1.1738x; 1.1738x over previous
"""Trainium2 Bass kernel for nn_AdaptiveSampler (sparse grid_sample attention).

Strategy (data-parallel over batch, 8 cores x 4 batch items each):
  - Host: features reshaped channels-last [B*H*W, C] in bf16 so every
    spatial cell is one contiguous 2KB row -> indirect row gathers.
  - Device per core:
      phase A: keypoint -> bilinear corner cells/weights (DVE f32 math)
      seed    = dma_gather(4 corners x 512 keypoints) -> weighted reduce
      MLPs    = PE matmuls (offsets + attention logits), softmax on DVE/ACT
      phase B: per keypoint a 4x4 patch around the seed cell covers all
               16 sample corners; per-cell weights are built by position
               selects (d = floor(px) - patch_base), folding attention
               softmax + bilinear + border validity into one weight.
      fused   = dma_gather of 8KB patch rows (4 x-cells, 4 rows/keypoint)
                * broadcast weights, segment-reduce, PE-transpose, DMA out.
All computation (gathers, MLPs, softmax, bilinear) happens on-device; the
host only reorders input layout and concatenates per-core outputs.
"""

import os
import sys
from contextlib import ExitStack

import numpy as np

sys.path.insert(0, "/opt/trn_rl_repo")

import ml_dtypes

import concourse.bass as bass
import concourse.tile as tile
from concourse import bacc, mybir

F32 = mybir.dt.float32
BF16 = mybir.dt.bfloat16
I16 = mybir.dt.int16

ALU = mybir.AluOpType
ACT = mybir.ActivationFunctionType
AX = mybir.AxisListType

B = 4          # batch items per core
C = 1024       # channels
H = W = 64
HW = H * W     # 4096 cells per batch item
J = 128        # keypoints
NP = 4         # sample points per keypoint
Q = C // 128   # 8 channel chunks
NIDX = J * 16  # 2048 indices per gather set (seed corners / main patch rows)
TWO23 = float(2 ** 23)


def _floor(nc, pool, src, shape, tag):
    """floor(src) on DVE via round-to-nearest + correction. Returns tile."""
    rnd = pool.tile(list(shape), F32, tag=f"floor_rnd_{tag}")
    nc.vector.tensor_scalar(rnd[:], src, TWO23, TWO23, ALU.add, ALU.subtract)
    flo = pool.tile(list(shape), F32, tag=f"floor_out_{tag}")
    nc.vector.tensor_tensor(flo[:], src, rnd[:], ALU.is_lt)
    nc.vector.tensor_tensor(flo[:], rnd[:], flo[:], ALU.subtract)
    return flo


def build_nc():
    nc = bacc.Bacc()

    feat = nc.declare_dram_parameter("feat", [B * HW, C], BF16, isOutput=False)
    kp = nc.declare_dram_parameter("kp", [J, 2 * B], F32, isOutput=False)
    w1o = nc.declare_dram_parameter("w1o", [128, Q, 128], BF16, isOutput=False)
    w1a = nc.declare_dram_parameter("w1a", [128, Q, 128], BF16, isOutput=False)
    w2o = nc.declare_dram_parameter("w2o", [128, 8], BF16, isOutput=False)
    w2a = nc.declare_dram_parameter("w2a", [128, 4], BF16, isOutput=False)
    b1o = nc.declare_dram_parameter("b1o", [128, 1], F32, isOutput=False)
    b1a = nc.declare_dram_parameter("b1a", [128, 1], F32, isOutput=False)
    b2o = nc.declare_dram_parameter("b2o", [8, 1], F32, isOutput=False)
    b2a = nc.declare_dram_parameter("b2a", [4, 1], F32, isOutput=False)
    bbase = nc.declare_dram_parameter("bbase", [128, B], F32, isOutput=False)
    posc = nc.declare_dram_parameter("posc", [128, 4], F32, isOutput=False)
    ident = nc.declare_dram_parameter("ident", [128, 128], F32, isOutput=False)
    identb = nc.declare_dram_parameter("identb", [128, 128], BF16, isOutput=False)
    out = nc.declare_dram_parameter("out", [B * J, C], BF16, isOutput=True)

    # DRAM scratch for flattening per-column weights before partition bcast
    wscr_s = nc.dram_tensor("wscr_s", [J * 16], BF16)
    wscr_m = nc.dram_tensor("wscr_m", [J * 64], BF16)

    # Overlapping row view of feat: row i = cells i..i+3 (8KB), for patch
    # gathers. Max row start 16380 -> read end == tensor end exactly.
    feat_ov = bass.AP(feat[:].tensor, 0, [[C, B * HW - 3], [1, 4 * C]])

    with ExitStack() as ctx:
        tc = ctx.enter_context(tile.TileContext(nc))
        cons = ctx.enter_context(tc.tile_pool(name="cons", bufs=1))
        a = ctx.enter_context(tc.tile_pool(name="phaseA", bufs=1))
        gp = ctx.enter_context(tc.tile_pool(name="gather", bufs=3))
        wp = ctx.enter_context(tc.tile_pool(name="wbc", bufs=1))
        op = ctx.enter_context(tc.tile_pool(name="outT", bufs=2))
        ip = ctx.enter_context(tc.tile_pool(name="idxw", bufs=2))
        ps = ctx.enter_context(tc.tile_pool(name="psT", bufs=3, space="PSUM"))
        pmm = ctx.enter_context(tc.tile_pool(name="psMM", bufs=2, space="PSUM"))

        # ---------------- constants ----------------
        def c_load(name, shape, dt, src):
            t = cons.tile(shape, dt, tag=name)
            nc.sync.dma_start(out=t[:], in_=src)
            return t

        kp_sb = c_load("kp", [J, B, 2], F32, kp[:].rearrange("j (b t) -> j b t", t=2))
        w1o_sb = c_load("w1o", [128, Q, 128], BF16, w1o[:])
        w1a_sb = c_load("w1a", [128, Q, 128], BF16, w1a[:])
        w2o_sb = c_load("w2o", [128, 8], BF16, w2o[:])
        w2a_sb = c_load("w2a", [128, 4], BF16, w2a[:])
        b1o_sb = c_load("b1o", [128, 1], F32, b1o[:])
        b1a_sb = c_load("b1a", [128, 1], F32, b1a[:])
        b2o_sb = c_load("b2o", [8, 1], F32, b2o[:])
        b2a_sb = c_load("b2a", [4, 1], F32, b2a[:])
        bbase_sb = c_load("bbase", [128, B], F32, bbase[:])
        posc_sb = c_load("posc", [128, 4], F32, posc[:])
        id_sb = c_load("ident", [128, 128], F32, ident[:])
        idb_sb = c_load("identb", [128, 128], BF16, identb[:])

        # ---------------- phase A: seed corners ----------------
        ix = a.tile([J, B], F32)
        nc.vector.tensor_scalar(ix[:], kp_sb[:, :, 0], 31.5, 31.5, ALU.mult, ALU.add)
        iy = a.tile([J, B], F32)
        nc.vector.tensor_scalar(iy[:], kp_sb[:, :, 1], 31.5, 31.5, ALU.mult, ALU.add)

        x0 = _floor(nc, a, ix[:], (J, B), "x0")
        y0 = _floor(nc, a, iy[:], (J, B), "y0")

        def pair_and_weights(base, i_coord, tagp):
            p = a.tile([J, B, 2], F32, tag=f"{tagp}_p")
            wgt = a.tile([J, B, 2], F32, tag=f"{tagp}_w")
            nc.vector.tensor_copy(p[:, :, 0], base[:])
            nc.vector.tensor_scalar_add(p[:, :, 1], base[:], 1.0)
            nc.vector.tensor_tensor(wgt[:, :, 1], i_coord, base[:], ALU.subtract)
            nc.vector.tensor_scalar(
                wgt[:, :, 0], wgt[:, :, 1], -1.0, 1.0, ALU.mult, ALU.add
            )
            return p, wgt

        xp, wxp = pair_and_weights(x0, ix[:], "x")
        yp, wyp = pair_and_weights(y0, iy[:], "y")

        # seed cell idx [J, B, 2cy, 2cx] = bbase + yp*64 + xp
        idx4 = a.tile([J, B, 2, 2], F32)
        t1 = a.tile([J, B, 2], F32)
        nc.vector.tensor_scalar_mul(t1[:], yp[:], 64.0)
        nc.vector.tensor_tensor(
            idx4[:],
            t1[:].unsqueeze(3).to_broadcast((J, B, 2, 2)),
            xp[:].unsqueeze(2).to_broadcast((J, B, 2, 2)),
            ALU.add,
        )
        nc.vector.tensor_tensor(
            idx4[:],
            idx4[:],
            bbase_sb[:].unsqueeze(2).unsqueeze(3).to_broadcast((J, B, 2, 2)),
            ALU.add,
        )
        w4 = a.tile([J, B, 2, 2], F32)
        nc.vector.tensor_tensor(
            w4[:],
            wyp[:].unsqueeze(3).to_broadcast((J, B, 2, 2)),
            wxp[:].unsqueeze(2).to_broadcast((J, B, 2, 2)),
            ALU.mult,
        )

        def wrap_idx(idx_flat_ap):
            """[J,16] f32 cell ids -> wrapped+replicated [128, J] int16 tile."""
            rep = ip.tile([J, 8, 16], F32, tag="idxrep")
            for g in range(8):
                nc.vector.tensor_copy(rep[:, g, :], idx_flat_ap)
            psT = ps.tile([128, J], F32, tag="tp")
            nc.tensor.transpose(
                psT[:], rep[:].rearrange("j g c -> j (g c)"), id_sb[:, :J]
            )
            idxw = ip.tile([128, J], I16, tag="idxw")
            nc.vector.tensor_copy(idxw[:], psT[:])
            return idxw

        def bcast_weights(w_flat_ap, wscr, n, slot):
            """[J, n] f32 col-weights -> [128, J*n] bf16 via DRAM bounce."""
            wb16 = a.tile([J, n], BF16, tag=f"wb16_{slot}")
            nc.vector.tensor_copy(wb16[:], w_flat_ap)
            nc.sync.dma_start(
                out=wscr[:].rearrange("(j c) -> j c", c=n), in_=wb16[:]
            )
            wbc = wp.tile([128, J * n], BF16, tag=f"wbc_{slot}")
            nc.sync.dma_start(
                out=wbc[:],
                in_=wscr[:].unsqueeze(0).to_broadcast((128, J * n)),
            )
            return wbc

        idxw_seed = wrap_idx(idx4[:].rearrange("j b cy cx -> j (b cy cx)"))
        wbc_seed = bcast_weights(
            w4[:].rearrange("j b cy cx -> j (b cy cx)"), wscr_s, 16, "s"
        )

        # ---------------- seed gather + combine ----------------
        HN = 512  # seed chunk: 512 idx x 2KB rows (desc-ring limit)
        seed = a.tile([128, Q, J * B], BF16)
        for h in range(4):
            seedg = gp.tile([128, Q, HN], BF16, tag="seedg")
            nc.gpsimd.dma_gather(
                seedg[:],
                feat[:],
                idxw_seed[:, 32 * h : 32 * h + 32],
                num_idxs=HN,
                num_idxs_reg=HN,
                elem_size=C,
                transpose=True,
            )
            with nc.allow_low_precision("bf16 grid-sample compute"):
                nc.vector.tensor_tensor(
                    seedg[:],
                    seedg[:],
                    wbc_seed[:, HN * h : HN * (h + 1)]
                    .unsqueeze(1)
                    .to_broadcast((128, Q, HN)),
                    ALU.mult,
                )
                nc.vector.tensor_reduce(
                    seed[:, :, 128 * h : 128 * (h + 1)],
                    seedg[:].rearrange("p q (jb c) -> p (q jb) c", c=4),
                    AX.X,
                    ALU.add,
                )

        # ---------------- MLPs ----------------
        def mlp_head(w1_sb, b1_sb, name):
            hps = pmm.tile([128, J * B], F32, tag="mm")
            for q in range(Q):
                nc.tensor.matmul(
                    hps[:],
                    w1_sb[:, q, :],
                    seed[:, q, :],
                    start=(q == 0),
                    stop=(q == Q - 1),
                )
            h_sb = a.tile([128, J * B], BF16, tag=f"hsb_{name}")
            nc.scalar.activation(h_sb[:], hps[:], ACT.Relu, bias=b1_sb[:])
            return h_sb

        h_off = mlp_head(w1o_sb, b1o_sb, "off")
        h_att = mlp_head(w1a_sb, b1a_sb, "att")

        ops2 = pmm.tile([8, J * B], F32, tag="mm")
        nc.tensor.matmul(ops2[:], w2o_sb[:], h_off[:], start=True, stop=True)
        off2 = a.tile([8, J * B], F32)
        nc.scalar.activation(off2[:], ops2[:], ACT.Identity, bias=b2o_sb[:])

        aps2 = pmm.tile([4, J * B], F32, tag="mm")
        nc.tensor.matmul(aps2[:], w2a_sb[:], h_att[:], start=True, stop=True)
        att2 = a.tile([4, J * B], F32)
        nc.scalar.activation(att2[:], aps2[:], ACT.Identity, bias=b2a_sb[:])

        # transpose MLP outputs back to [J, B, ch] layout (per-b strided cols)
        offT = a.tile([J, B, 8], F32)
        attT = a.tile([J, B, 4], F32)
        for b in range(B):
            pso = ps.tile([J, 8], F32, tag="tp")
            nc.tensor.transpose(pso[:], off2[:, b::B], id_sb[:8, :8])
            nc.scalar.copy(offT[:, b, :], pso[:])
            psa = ps.tile([J, 4], F32, tag="tp")
            nc.tensor.transpose(psa[:], att2[:, b::B], id_sb[:4, :4])
            nc.scalar.copy(attT[:, b, :], psa[:])

        # ---------------- phase B: 4x4 patch per keypoint ----------------
        # patch base bx/by [J, B] = clip(seed_corner - 1, 0, 60)
        bx = a.tile([J, B], F32)
        nc.vector.tensor_scalar(bx[:], x0[:], -1.0, 0.0, ALU.add, ALU.max)
        nc.vector.tensor_scalar_min(bx[:], bx[:], 60.0)
        by = a.tile([J, B], F32)
        nc.vector.tensor_scalar(by[:], y0[:], -1.0, 0.0, ALU.add, ALU.max)
        nc.vector.tensor_scalar_min(by[:], by[:], 60.0)

        # per-point coords px/py [J, B, NP]
        px = a.tile([J, B, NP], F32)
        nc.vector.tensor_tensor(
            px[:],
            ix[:].unsqueeze(2).to_broadcast((J, B, NP)),
            offT[:, :, 0:NP],
            ALU.add,
        )
        py = a.tile([J, B, NP], F32)
        nc.vector.tensor_tensor(
            py[:],
            iy[:].unsqueeze(2).to_broadcast((J, B, NP)),
            offT[:, :, NP : 2 * NP],
            ALU.add,
        )

        # softmax over NP  [J, B, NP]
        amax = a.tile([J, B, 1], F32)
        nc.vector.tensor_reduce(amax[:], attT[:], AX.X, ALU.max)
        ae = a.tile([J, B, NP], F32)
        nc.vector.tensor_tensor(
            ae[:], attT[:], amax[:].to_broadcast((J, B, NP)), ALU.subtract
        )
        nc.scalar.activation(ae[:], ae[:], ACT.Exp)
        asum = a.tile([J, B, 1], F32)
        nc.vector.tensor_reduce(asum[:], ae[:], AX.X, ALU.add)
        nc.vector.reciprocal(asum[:], asum[:])
        attw = a.tile([J, B, NP], F32)
        nc.vector.tensor_tensor(
            attw[:], ae[:], asum[:].to_broadcast((J, B, NP)), ALU.mult
        )

        def axis_select(pc, base, tagp):
            """Position-select weights [J, B, NP, 4pos]:
            w0*(pos==d) + w1*(pos==d+1), d = floor(pc) - base."""
            c0 = _floor(nc, a, pc[:], (J, B, NP), tagp)
            w1t = a.tile([J, B, NP], F32, tag=f"{tagp}_w1")
            nc.vector.tensor_tensor(w1t[:], pc[:], c0[:], ALU.subtract)
            w0t = a.tile([J, B, NP], F32, tag=f"{tagp}_w0")
            nc.vector.tensor_scalar(w0t[:], w1t[:], -1.0, 1.0, ALU.mult, ALU.add)
            d = a.tile([J, B, NP], F32, tag=f"{tagp}_d")
            nc.vector.tensor_tensor(
                d[:], c0[:], base[:].unsqueeze(2).to_broadcast((J, B, NP)),
                ALU.subtract,
            )
            d1 = a.tile([J, B, NP], F32, tag=f"{tagp}_d1")
            nc.vector.tensor_scalar_add(d1[:], d[:], 1.0)
            posb = posc_sb[:].unsqueeze(1).unsqueeze(2).to_broadcast((J, B, NP, 4))
            sel = a.tile([J, B, NP, 4], F32, tag=f"{tagp}_sel")
            eq = a.tile([J, B, NP, 4], F32, tag=f"{tagp}_eq")
            nc.vector.tensor_tensor(
                eq[:], d[:].unsqueeze(3).to_broadcast((J, B, NP, 4)), posb,
                ALU.is_equal,
            )
            nc.vector.tensor_tensor(
                sel[:], eq[:], w0t[:].unsqueeze(3).to_broadcast((J, B, NP, 4)),
                ALU.mult,
            )
            nc.vector.tensor_tensor(
                eq[:], d1[:].unsqueeze(3).to_broadcast((J, B, NP, 4)), posb,
                ALU.is_equal,
            )
            nc.vector.tensor_tensor(
                eq[:], eq[:], w1t[:].unsqueeze(3).to_broadcast((J, B, NP, 4)),
                ALU.mult,
            )
            nc.vector.tensor_tensor(sel[:], sel[:], eq[:], ALU.add)
            return sel

        wxsel = axis_select(px, bx, "sx")
        wysel = axis_select(py, by, "sy")

        # fold attention weight into y-selects: ty [J, B, NP, 4Y]
        ty = a.tile([J, B, NP, 4], F32)
        nc.vector.tensor_tensor(
            ty[:], wysel[:], attw[:].unsqueeze(3).to_broadcast((J, B, NP, 4)),
            ALU.mult,
        )
        # patch weights w44 [J, B, 4Y, 4X] = sum_n ty[n, Y] * wxsel[n, X]
        w44 = a.tile([J, B, 4, 4], F32)
        tmp44 = a.tile([J, B, 4, 4], F32)
        for n in range(NP):
            dst = w44 if n == 0 else tmp44
            nc.vector.tensor_tensor(
                dst[:],
                ty[:, :, n, :].unsqueeze(3).to_broadcast((J, B, 4, 4)),
                wxsel[:, :, n, :].unsqueeze(2).to_broadcast((J, B, 4, 4)),
                ALU.mult,
            )
            if n > 0:
                nc.vector.tensor_tensor(w44[:], w44[:], tmp44[:], ALU.add)

        # patch row ids [J, B, 4Y] = bbase + (by + Y)*64 + bx
        pbase = a.tile([J, B], F32)
        nc.vector.tensor_scalar_mul(pbase[:], by[:], 64.0)
        nc.vector.tensor_tensor(pbase[:], pbase[:], bx[:], ALU.add)
        nc.vector.tensor_tensor(pbase[:], pbase[:], bbase_sb[:], ALU.add)
        y64 = a.tile([128, 4], F32)
        nc.vector.tensor_scalar_mul(y64[:], posc_sb[:], 64.0)
        idxp = a.tile([J, B, 4], F32)
        nc.vector.tensor_tensor(
            idxp[:],
            pbase[:].unsqueeze(2).to_broadcast((J, B, 4)),
            y64[:].unsqueeze(1).to_broadcast((J, B, 4)),
            ALU.add,
        )

        idxw_m = wrap_idx(idxp[:].rearrange("j b y -> j (b y)"))
        wbc_m = bcast_weights(
            w44[:].rearrange("j b y x -> j (b y x)"), wscr_m, 64, "m"
        )

        # ---------------- main gather + fuse (16 chunks of 8 keypoints) ----
        MN = 128  # idxs per chunk: 8 keypoints x (4b x 4Y) 8KB patch rows
        fused = a.tile([128, Q, J * B], BF16)
        with nc.allow_low_precision("bf16 grid-sample compute"):
            for h in range(16):
                mg = gp.tile([128, 4 * Q, MN], BF16, tag="mg")
                nc.gpsimd.dma_gather(
                    mg[:],
                    feat_ov,
                    idxw_m[:, 8 * h : 8 * h + 8],
                    num_idxs=MN,
                    num_idxs_reg=MN,
                    elem_size=4 * C,
                    elem_step=C,
                    transpose=True,
                )
                nc.vector.tensor_tensor(
                    mg[:].rearrange("p (x q) i -> p x q i", x=4),
                    mg[:].rearrange("p (x q) i -> p x q i", x=4),
                    wbc_m[:, 512 * h : 512 * (h + 1)]
                    .rearrange("p (jby x) -> p x jby", x=4)
                    .unsqueeze(2)
                    .to_broadcast((128, 4, Q, MN)),
                    ALU.mult,
                )
                m5 = mg[:].rearrange(
                    "p (x q) (jb y) -> p x (q jb) y", x=4, y=4
                )
                fsl = fused[:, :, 32 * h : 32 * h + 32]
                nc.vector.tensor_reduce(fsl, m5[:, 0, :, :], AX.X, ALU.add)
                xacc = gp.tile([128, Q, 32], BF16, tag="xacc")
                for x in range(1, 4):
                    nc.vector.tensor_reduce(
                        xacc[:], m5[:, x, :, :], AX.X, ALU.add
                    )
                    nc.vector.tensor_tensor(fsl, fsl, xacc[:], ALU.add)

        # ---------------- output transpose + store ----------------
        for b in range(B):
            outT = op.tile([J, Q, 128], BF16, tag="outT")
            for q in range(Q):
                pst = ps.tile([J, 128], BF16, tag="tpb")
                nc.tensor.transpose(pst[:], fused[:, q, b::B], idb_sb[:, :J])
                nc.scalar.copy(outT[:, q, :], pst[:])
            nc.sync.dma_start(
                out=out[b * J : (b + 1) * J, :].rearrange(
                    "j (q c) -> j q c", q=Q
                ),
                in_=outT[:],
            )

    nc.finalize()
    return nc


def prepare_in_maps(features, keypoint_coords, w_off1, b_off1, w_off2, b_off2,
                    w_att1, b_att1, w_att2, b_att2, n_cores=8):
    bf = ml_dtypes.bfloat16
    f32 = np.float32

    def w1t(w):  # [128, C] -> [128 k_local, Q, 128 m] bf16
        return np.ascontiguousarray(
            w.T.reshape(Q, 128, 128).transpose(1, 0, 2).astype(bf)
        )

    w1o_h = w1t(np.asarray(w_off1, f32))
    w1a_h = w1t(np.asarray(w_att1, f32))
    w2o_h = np.ascontiguousarray(
        np.concatenate([w_off2[0::2], w_off2[1::2]], 0).T.astype(bf)
    )
    w2a_h = np.ascontiguousarray(np.asarray(w_att2, f32).T.astype(bf))
    b1o_h = np.asarray(b_off1, f32).reshape(128, 1).copy()
    b1a_h = np.asarray(b_att1, f32).reshape(128, 1).copy()
    b2o_h = np.concatenate([b_off2[0::2], b_off2[1::2]]).astype(f32).reshape(8, 1)
    b2a_h = np.asarray(b_att2, f32).reshape(4, 1).copy()
    bbase_h = np.broadcast_to(
        (np.arange(B, dtype=f32) * HW)[None, :], (128, B)
    ).copy()
    posc_h = np.broadcast_to(np.arange(4, dtype=f32)[None, :], (128, 4)).copy()
    ident_h = np.eye(128, dtype=f32)
    identb_h = np.eye(128, dtype=f32).astype(bf)

    in_maps = []
    for m in range(n_cores):
        bs = slice(B * m, B * (m + 1))
        feat_h = np.ascontiguousarray(
            np.asarray(features[bs], f32).transpose(0, 2, 3, 1).reshape(B * HW, C)
        ).astype(bf)
        kp_h = np.ascontiguousarray(
            np.asarray(keypoint_coords[bs], f32).transpose(1, 0, 2).reshape(J, 2 * B)
        )
        in_maps.append({
            "feat": feat_h, "kp": kp_h,
            "w1o": w1o_h, "w1a": w1a_h, "w2o": w2o_h, "w2a": w2a_h,
            "b1o": b1o_h, "b1a": b1a_h, "b2o": b2o_h, "b2a": b2a_h,
            "bbase": bbase_h, "posc": posc_h,
            "ident": ident_h, "identb": identb_h,
        })
    return in_maps


_NC_CACHE = None


def get_nc():
    global _NC_CACHE
    if _NC_CACHE is None:
        _NC_CACHE = build_nc()
    return _NC_CACHE


def kernel(**inputs):
    from concourse.bass_utils import run_bass_kernel_spmd

    n_cores = 8
    nc = get_nc()
    in_maps = prepare_in_maps(**inputs, n_cores=n_cores)
    res = run_bass_kernel_spmd(
        nc, in_maps, core_ids=list(range(n_cores)),
        trace=bool(int(os.environ.get("KERNEL_TRACE", "0") or 0)),
    )
    kernel.last_results = res
    outs = [
        np.asarray(r["out"]).astype(np.float32).reshape(B, J, C)
        for r in res.results
    ]
    return np.concatenate(outs, axis=0)


# revision 13
# speedup vs baseline: 1.1764x; 1.0022x over previous
"""Trainium2 Bass kernel for nn_AdaptiveSampler (sparse grid_sample attention).

Strategy (data-parallel over batch, 8 cores x 4 batch items each):
  - Host: features reshaped channels-last [B*H*W, C] in bf16 so every
    spatial cell is one contiguous 2KB row -> indirect row gathers.
  - Device per core:
      phase A: keypoint -> bilinear corner cells/weights (DVE f32 math)
      seed    = dma_gather(4 corners x 512 keypoints) -> weighted reduce
      MLPs    = PE matmuls (offsets + attention logits), softmax on DVE/ACT
      phase B: per keypoint a 4x4 patch around the seed cell covers all
               16 sample corners; per-cell weights are built by position
               selects (d = floor(px) - patch_base), folding attention
               softmax + bilinear + border validity into one weight.
      fused   = dma_gather of 8KB patch rows (4 x-cells, 4 rows/keypoint)
                * broadcast weights, segment-reduce, PE-transpose, DMA out.
All computation (gathers, MLPs, softmax, bilinear) happens on-device; the
host only reorders input layout and concatenates per-core outputs.
"""

import os
import sys
from contextlib import ExitStack

import numpy as np

sys.path.insert(0, "/opt/trn_rl_repo")

import ml_dtypes

import concourse.bass as bass
import concourse.tile as tile
from concourse import bacc, mybir

F32 = mybir.dt.float32
BF16 = mybir.dt.bfloat16
I16 = mybir.dt.int16

ALU = mybir.AluOpType
ACT = mybir.ActivationFunctionType
AX = mybir.AxisListType

B = 4          # batch items per core
C = 1024       # channels
H = W = 64
HW = H * W     # 4096 cells per batch item
J = 128        # keypoints
NP = 4         # sample points per keypoint
Q = C // 128   # 8 channel chunks
NIDX = J * 16  # 2048 indices per gather set (seed corners / main patch rows)
TWO23 = float(2 ** 23)


def _floor(nc, pool, src, shape, tag):
    """floor(src) on DVE via round-to-nearest + correction. Returns tile."""
    rnd = pool.tile(list(shape), F32, tag=f"floor_rnd_{tag}")
    nc.vector.tensor_scalar(rnd[:], src, TWO23, TWO23, ALU.add, ALU.subtract)
    flo = pool.tile(list(shape), F32, tag=f"floor_out_{tag}")
    nc.vector.tensor_tensor(flo[:], src, rnd[:], ALU.is_lt)
    nc.vector.tensor_tensor(flo[:], rnd[:], flo[:], ALU.subtract)
    return flo


def build_nc():
    nc = bacc.Bacc()

    feat = nc.declare_dram_parameter("feat", [B * HW, C], BF16, isOutput=False)
    kp = nc.declare_dram_parameter("kp", [J, 2 * B], F32, isOutput=False)
    w1o = nc.declare_dram_parameter("w1o", [128, Q, 128], BF16, isOutput=False)
    w1a = nc.declare_dram_parameter("w1a", [128, Q, 128], BF16, isOutput=False)
    w2o = nc.declare_dram_parameter("w2o", [128, 8], BF16, isOutput=False)
    w2a = nc.declare_dram_parameter("w2a", [128, 4], BF16, isOutput=False)
    b1o = nc.declare_dram_parameter("b1o", [128, 1], F32, isOutput=False)
    b1a = nc.declare_dram_parameter("b1a", [128, 1], F32, isOutput=False)
    b2o = nc.declare_dram_parameter("b2o", [8, 1], F32, isOutput=False)
    b2a = nc.declare_dram_parameter("b2a", [4, 1], F32, isOutput=False)
    bbase = nc.declare_dram_parameter("bbase", [128, B], F32, isOutput=False)
    posc = nc.declare_dram_parameter("posc", [128, 4], F32, isOutput=False)
    ident = nc.declare_dram_parameter("ident", [128, 128], F32, isOutput=False)
    identb = nc.declare_dram_parameter("identb", [128, 128], BF16, isOutput=False)
    out = nc.declare_dram_parameter("out", [B * J, C], BF16, isOutput=True)

    # DRAM scratch for flattening per-column weights before partition bcast
    wscr_s = nc.dram_tensor("wscr_s", [J * 16], BF16)
    wscr_m = nc.dram_tensor("wscr_m", [J * 64], BF16)

    # Overlapping row view of feat: row i = cells i..i+3 (8KB), for patch
    # gathers. Max row start 16380 -> read end == tensor end exactly.
    feat_ov = bass.AP(feat[:].tensor, 0, [[C, B * HW - 3], [1, 4 * C]])

    with ExitStack() as ctx:
        tc = ctx.enter_context(tile.TileContext(nc))
        cons = ctx.enter_context(tc.tile_pool(name="cons", bufs=1))
        a = ctx.enter_context(tc.tile_pool(name="phaseA", bufs=1))
        gp = ctx.enter_context(tc.tile_pool(name="gather", bufs=3))
        wp = ctx.enter_context(tc.tile_pool(name="wbc", bufs=1))
        op = ctx.enter_context(tc.tile_pool(name="outT", bufs=2))
        ip = ctx.enter_context(tc.tile_pool(name="idxw", bufs=2))
        ps = ctx.enter_context(tc.tile_pool(name="psT", bufs=3, space="PSUM"))
        pmm = ctx.enter_context(tc.tile_pool(name="psMM", bufs=2, space="PSUM"))

        # ---------------- constants ----------------
        def c_load(name, shape, dt, src):
            t = cons.tile(shape, dt, tag=name)
            nc.sync.dma_start(out=t[:], in_=src)
            return t

        kp_sb = c_load("kp", [J, B, 2], F32, kp[:].rearrange("j (b t) -> j b t", t=2))
        w1o_sb = c_load("w1o", [128, Q, 128], BF16, w1o[:])
        w1a_sb = c_load("w1a", [128, Q, 128], BF16, w1a[:])
        w2o_sb = c_load("w2o", [128, 8], BF16, w2o[:])
        w2a_sb = c_load("w2a", [128, 4], BF16, w2a[:])
        b1o_sb = c_load("b1o", [128, 1], F32, b1o[:])
        b1a_sb = c_load("b1a", [128, 1], F32, b1a[:])
        b2o_sb = c_load("b2o", [8, 1], F32, b2o[:])
        b2a_sb = c_load("b2a", [4, 1], F32, b2a[:])
        bbase_sb = c_load("bbase", [128, B], F32, bbase[:])
        posc_sb = c_load("posc", [128, 4], F32, posc[:])
        id_sb = c_load("ident", [128, 128], F32, ident[:])
        idb_sb = c_load("identb", [128, 128], BF16, identb[:])

        # ---------------- phase A: seed corners ----------------
        ix = a.tile([J, B], F32)
        nc.vector.tensor_scalar(ix[:], kp_sb[:, :, 0], 31.5, 31.5, ALU.mult, ALU.add)
        iy = a.tile([J, B], F32)
        nc.vector.tensor_scalar(iy[:], kp_sb[:, :, 1], 31.5, 31.5, ALU.mult, ALU.add)

        x0 = _floor(nc, a, ix[:], (J, B), "x0")
        y0 = _floor(nc, a, iy[:], (J, B), "y0")

        def pair_and_weights(base, i_coord, tagp):
            p = a.tile([J, B, 2], F32, tag=f"{tagp}_p")
            wgt = a.tile([J, B, 2], F32, tag=f"{tagp}_w")
            nc.vector.tensor_copy(p[:, :, 0], base[:])
            nc.vector.tensor_scalar_add(p[:, :, 1], base[:], 1.0)
            nc.vector.tensor_tensor(wgt[:, :, 1], i_coord, base[:], ALU.subtract)
            nc.vector.tensor_scalar(
                wgt[:, :, 0], wgt[:, :, 1], -1.0, 1.0, ALU.mult, ALU.add
            )
            return p, wgt

        xp, wxp = pair_and_weights(x0, ix[:], "x")
        yp, wyp = pair_and_weights(y0, iy[:], "y")

        # seed cell idx [J, B, 2cy, 2cx] = bbase + yp*64 + xp
        idx4 = a.tile([J, B, 2, 2], F32)
        t1 = a.tile([J, B, 2], F32)
        nc.vector.tensor_scalar_mul(t1[:], yp[:], 64.0)
        nc.vector.tensor_tensor(
            idx4[:],
            t1[:].unsqueeze(3).to_broadcast((J, B, 2, 2)),
            xp[:].unsqueeze(2).to_broadcast((J, B, 2, 2)),
            ALU.add,
        )
        nc.vector.tensor_tensor(
            idx4[:],
            idx4[:],
            bbase_sb[:].unsqueeze(2).unsqueeze(3).to_broadcast((J, B, 2, 2)),
            ALU.add,
        )
        w4 = a.tile([J, B, 2, 2], F32)
        nc.vector.tensor_tensor(
            w4[:],
            wyp[:].unsqueeze(3).to_broadcast((J, B, 2, 2)),
            wxp[:].unsqueeze(2).to_broadcast((J, B, 2, 2)),
            ALU.mult,
        )

        def wrap_idx(idx_flat_ap):
            """[J,16] f32 cell ids -> wrapped+replicated [128, J] int16 tile."""
            rep = ip.tile([J, 8, 16], F32, tag="idxrep")
            for g in range(8):
                nc.vector.tensor_copy(rep[:, g, :], idx_flat_ap)
            psT = ps.tile([128, J], F32, tag="tp")
            nc.tensor.transpose(
                psT[:], rep[:].rearrange("j g c -> j (g c)"), id_sb[:, :J]
            )
            idxw = ip.tile([128, J], I16, tag="idxw")
            nc.vector.tensor_copy(idxw[:], psT[:])
            return idxw

        def bcast_weights(w_flat_ap, wscr, n, slot, dest_view=None):
            """[J, n] f32 col-weights -> [128, J*n] bf16 via DRAM bounce.
            dest_view(wscr_ap) may reorder the DRAM layout."""
            wb16 = a.tile([J, n], BF16, tag=f"wb16_{slot}")
            nc.vector.tensor_copy(wb16[:], w_flat_ap)
            dst = (
                dest_view(wscr[:])
                if dest_view is not None
                else wscr[:].rearrange("(j c) -> j c", c=n)
            )
            nc.sync.dma_start(out=dst, in_=wb16[:])
            wbc = wp.tile([128, J * n], BF16, tag=f"wbc_{slot}")
            nc.sync.dma_start(
                out=wbc[:],
                in_=wscr[:].unsqueeze(0).to_broadcast((128, J * n)),
            )
            return wbc

        idxw_seed = wrap_idx(idx4[:].rearrange("j b cy cx -> j (b cy cx)"))
        wbc_seed = bcast_weights(
            w4[:].rearrange("j b cy cx -> j (b cy cx)"), wscr_s, 16, "s"
        )

        # ---------------- seed gather + combine ----------------
        HN = 512  # seed chunk: 512 idx x 2KB rows (desc-ring limit)
        seed = a.tile([128, Q, J * B], BF16)
        for h in range(4):
            seedg = gp.tile([128, Q, HN], BF16, tag="seedg")
            nc.gpsimd.dma_gather(
                seedg[:],
                feat[:],
                idxw_seed[:, 32 * h : 32 * h + 32],
                num_idxs=HN,
                num_idxs_reg=HN,
                elem_size=C,
                transpose=True,
            )
            with nc.allow_low_precision("bf16 grid-sample compute"):
                nc.vector.tensor_tensor(
                    seedg[:],
                    seedg[:],
                    wbc_seed[:, HN * h : HN * (h + 1)]
                    .unsqueeze(1)
                    .to_broadcast((128, Q, HN)),
                    ALU.mult,
                )
                nc.vector.tensor_reduce(
                    seed[:, :, 128 * h : 128 * (h + 1)],
                    seedg[:].rearrange("p q (jb c) -> p (q jb) c", c=4),
                    AX.X,
                    ALU.add,
                )

        # ---------------- MLPs ----------------
        def mlp_head(w1_sb, b1_sb, name):
            hps = pmm.tile([128, J * B], F32, tag="mm")
            for q in range(Q):
                nc.tensor.matmul(
                    hps[:],
                    w1_sb[:, q, :],
                    seed[:, q, :],
                    start=(q == 0),
                    stop=(q == Q - 1),
                )
            h_sb = a.tile([128, J * B], BF16, tag=f"hsb_{name}")
            nc.scalar.activation(h_sb[:], hps[:], ACT.Relu, bias=b1_sb[:])
            return h_sb

        h_off = mlp_head(w1o_sb, b1o_sb, "off")
        h_att = mlp_head(w1a_sb, b1a_sb, "att")

        ops2 = pmm.tile([8, J * B], F32, tag="mm")
        nc.tensor.matmul(ops2[:], w2o_sb[:], h_off[:], start=True, stop=True)
        off2 = a.tile([8, J * B], F32)
        nc.scalar.activation(off2[:], ops2[:], ACT.Identity, bias=b2o_sb[:])

        aps2 = pmm.tile([4, J * B], F32, tag="mm")
        nc.tensor.matmul(aps2[:], w2a_sb[:], h_att[:], start=True, stop=True)
        att2 = a.tile([4, J * B], F32)
        nc.scalar.activation(att2[:], aps2[:], ACT.Identity, bias=b2a_sb[:])

        # transpose MLP outputs back to [J, B, ch] layout (per-b strided cols)
        offT = a.tile([J, B, 8], F32)
        attT = a.tile([J, B, 4], F32)
        for b in range(B):
            pso = ps.tile([J, 8], F32, tag="tp")
            nc.tensor.transpose(pso[:], off2[:, b::B], id_sb[:8, :8])
            nc.scalar.copy(offT[:, b, :], pso[:])
            psa = ps.tile([J, 4], F32, tag="tp")
            nc.tensor.transpose(psa[:], att2[:, b::B], id_sb[:4, :4])
            nc.scalar.copy(attT[:, b, :], psa[:])

        # ---------------- phase B: 4x4 patch per keypoint ----------------
        # patch base bx/by [J, B] = clip(seed_corner - 1, 0, 60)
        bx = a.tile([J, B], F32)
        nc.vector.tensor_scalar(bx[:], x0[:], -1.0, 0.0, ALU.add, ALU.max)
        nc.vector.tensor_scalar_min(bx[:], bx[:], 60.0)
        by = a.tile([J, B], F32)
        nc.vector.tensor_scalar(by[:], y0[:], -1.0, 0.0, ALU.add, ALU.max)
        nc.vector.tensor_scalar_min(by[:], by[:], 60.0)

        # per-point coords px/py [J, B, NP]
        px = a.tile([J, B, NP], F32)
        nc.vector.tensor_tensor(
            px[:],
            ix[:].unsqueeze(2).to_broadcast((J, B, NP)),
            offT[:, :, 0:NP],
            ALU.add,
        )
        py = a.tile([J, B, NP], F32)
        nc.vector.tensor_tensor(
            py[:],
            iy[:].unsqueeze(2).to_broadcast((J, B, NP)),
            offT[:, :, NP : 2 * NP],
            ALU.add,
        )

        # softmax over NP  [J, B, NP]
        amax = a.tile([J, B, 1], F32)
        nc.vector.tensor_reduce(amax[:], attT[:], AX.X, ALU.max)
        ae = a.tile([J, B, NP], F32)
        nc.vector.tensor_tensor(
            ae[:], attT[:], amax[:].to_broadcast((J, B, NP)), ALU.subtract
        )
        nc.scalar.activation(ae[:], ae[:], ACT.Exp)
        asum = a.tile([J, B, 1], F32)
        nc.vector.tensor_reduce(asum[:], ae[:], AX.X, ALU.add)
        nc.vector.reciprocal(asum[:], asum[:])
        attw = a.tile([J, B, NP], F32)
        nc.vector.tensor_tensor(
            attw[:], ae[:], asum[:].to_broadcast((J, B, NP)), ALU.mult
        )

        def axis_select(pc, base, tagp):
            """Position-select weights [J, B, NP, 4pos]:
            w0*(pos==d) + w1*(pos==d+1), d = floor(pc) - base."""
            c0 = _floor(nc, a, pc[:], (J, B, NP), tagp)
            w1t = a.tile([J, B, NP], F32, tag=f"{tagp}_w1")
            nc.vector.tensor_tensor(w1t[:], pc[:], c0[:], ALU.subtract)
            w0t = a.tile([J, B, NP], F32, tag=f"{tagp}_w0")
            nc.vector.tensor_scalar(w0t[:], w1t[:], -1.0, 1.0, ALU.mult, ALU.add)
            d = a.tile([J, B, NP], F32, tag=f"{tagp}_d")
            nc.vector.tensor_tensor(
                d[:], c0[:], base[:].unsqueeze(2).to_broadcast((J, B, NP)),
                ALU.subtract,
            )
            d1 = a.tile([J, B, NP], F32, tag=f"{tagp}_d1")
            nc.vector.tensor_scalar_add(d1[:], d[:], 1.0)
            posb = posc_sb[:].unsqueeze(1).unsqueeze(2).to_broadcast((J, B, NP, 4))
            sel = a.tile([J, B, NP, 4], F32, tag=f"{tagp}_sel")
            eq = a.tile([J, B, NP, 4], F32, tag=f"{tagp}_eq")
            nc.vector.tensor_tensor(
                eq[:], d[:].unsqueeze(3).to_broadcast((J, B, NP, 4)), posb,
                ALU.is_equal,
            )
            nc.vector.tensor_tensor(
                sel[:], eq[:], w0t[:].unsqueeze(3).to_broadcast((J, B, NP, 4)),
                ALU.mult,
            )
            nc.vector.tensor_tensor(
                eq[:], d1[:].unsqueeze(3).to_broadcast((J, B, NP, 4)), posb,
                ALU.is_equal,
            )
            nc.vector.tensor_tensor(
                eq[:], eq[:], w1t[:].unsqueeze(3).to_broadcast((J, B, NP, 4)),
                ALU.mult,
            )
            nc.vector.tensor_tensor(sel[:], sel[:], eq[:], ALU.add)
            return sel

        wxsel = axis_select(px, bx, "sx")
        wysel = axis_select(py, by, "sy")

        # fold attention weight into y-selects: ty [J, B, NP, 4Y]
        ty = a.tile([J, B, NP, 4], F32)
        nc.vector.tensor_tensor(
            ty[:], wysel[:], attw[:].unsqueeze(3).to_broadcast((J, B, NP, 4)),
            ALU.mult,
        )
        # patch weights w44 [J, B, 4Y, 4X] = sum_n ty[n, Y] * wxsel[n, X]
        w44 = a.tile([J, B, 4, 4], F32)
        tmp44 = a.tile([J, B, 4, 4], F32)
        for n in range(NP):
            dst = w44 if n == 0 else tmp44
            nc.vector.tensor_tensor(
                dst[:],
                ty[:, :, n, :].unsqueeze(3).to_broadcast((J, B, 4, 4)),
                wxsel[:, :, n, :].unsqueeze(2).to_broadcast((J, B, 4, 4)),
                ALU.mult,
            )
            if n > 0:
                nc.vector.tensor_tensor(w44[:], w44[:], tmp44[:], ALU.add)

        # patch row ids [J, B, 4Y] = bbase + (by + Y)*64 + bx
        pbase = a.tile([J, B], F32)
        nc.vector.tensor_scalar_mul(pbase[:], by[:], 64.0)
        nc.vector.tensor_tensor(pbase[:], pbase[:], bx[:], ALU.add)
        nc.vector.tensor_tensor(pbase[:], pbase[:], bbase_sb[:], ALU.add)
        y64 = a.tile([128, 4], F32)
        nc.vector.tensor_scalar_mul(y64[:], posc_sb[:], 64.0)
        idxp = a.tile([J, B, 4], F32)
        nc.vector.tensor_tensor(
            idxp[:],
            pbase[:].unsqueeze(2).to_broadcast((J, B, 4)),
            y64[:].unsqueeze(1).to_broadcast((J, B, 4)),
            ALU.add,
        )

        idxw_m = wrap_idx(idxp[:].rearrange("j b y -> j (b y)"))
        wbc_m = bcast_weights(
            w44[:].rearrange("j b y x -> j (b y x)"), wscr_m, 64, "m",
            dest_view=lambda ap: ap.rearrange(
                "(x j b y) -> j (b y) x", x=4, j=J, b=B
            ),
        )

        # ---------------- main gather + fuse (16 chunks of 8 keypoints) ----
        MN = 128  # idxs per chunk: 8 keypoints x (4b x 4Y) 8KB patch rows
        fused = a.tile([128, Q, J * B], BF16)
        with nc.allow_low_precision("bf16 grid-sample compute"):
            for h in range(16):
                mg = gp.tile([128, 4 * Q, MN], BF16, tag="mg")
                nc.gpsimd.dma_gather(
                    mg[:],
                    feat_ov,
                    idxw_m[:, 8 * h : 8 * h + 8],
                    num_idxs=MN,
                    num_idxs_reg=MN,
                    elem_size=4 * C,
                    elem_step=C,
                    transpose=True,
                )
                mv = mg[:].rearrange("p (x q) i -> p x q i", x=4)
                for x in range(4):
                    nc.vector.tensor_tensor(
                        mv[:, x, :, :],
                        mv[:, x, :, :],
                        wbc_m[:, 2048 * x + 128 * h : 2048 * x + 128 * h + 128]
                        .unsqueeze(1)
                        .to_broadcast((128, Q, MN)),
                        ALU.mult,
                    )
                # sum over x: two pairwise adds on contiguous views
                nc.vector.tensor_tensor(
                    mg[:, 0:16, :], mg[:, 0:16, :], mg[:, 16:32, :], ALU.add
                )
                nc.vector.tensor_tensor(
                    mg[:, 0:8, :], mg[:, 0:8, :], mg[:, 8:16, :], ALU.add
                )
                fsl = fused[:, :, 32 * h : 32 * h + 32]
                nc.vector.tensor_reduce(
                    fsl,
                    mg[:, 0:8, :].rearrange("p q (jb y) -> p (q jb) y", y=4),
                    AX.X,
                    ALU.add,
                )

        # ---------------- output transpose + store ----------------
        for b in range(B):
            outT = op.tile([J, Q, 128], BF16, tag="outT")
            for q in range(Q):
                pst = ps.tile([J, 128], BF16, tag="tpb")
                nc.tensor.transpose(pst[:], fused[:, q, b::B], idb_sb[:, :J])
                nc.scalar.copy(outT[:, q, :], pst[:])
            nc.sync.dma_start(
                out=out[b * J : (b + 1) * J, :].rearrange(
                    "j (q c) -> j q c", q=Q
                ),
                in_=outT[:],
            )

    nc.finalize()
    return nc


def prepare_in_maps(features, keypoint_coords, w_off1, b_off1, w_off2, b_off2,
                    w_att1, b_att1, w_att2, b_att2, n_cores=8):
    bf = ml_dtypes.bfloat16
    f32 = np.float32

    def w1t(w):  # [128, C] -> [128 k_local, Q, 128 m] bf16
        return np.ascontiguousarray(
            w.T.reshape(Q, 128, 128).transpose(1, 0, 2).astype(bf)
        )

    w1o_h = w1t(np.asarray(w_off1, f32))
    w1a_h = w1t(np.asarray(w_att1, f32))
    w2o_h = np.ascontiguousarray(
        np.concatenate([w_off2[0::2], w_off2[1::2]], 0).T.astype(bf)
    )
    w2a_h = np.ascontiguousarray(np.asarray(w_att2, f32).T.astype(bf))
    b1o_h = np.asarray(b_off1, f32).reshape(128, 1).copy()
    b1a_h = np.asarray(b_att1, f32).reshape(128, 1).copy()
    b2o_h = np.concatenate([b_off2[0::2], b_off2[1::2]]).astype(f32).reshape(8, 1)
    b2a_h = np.asarray(b_att2, f32).reshape(4, 1).copy()
    bbase_h = np.broadcast_to(
        (np.arange(B, dtype=f32) * HW)[None, :], (128, B)
    ).copy()
    posc_h = np.broadcast_to(np.arange(4, dtype=f32)[None, :], (128, 4)).copy()
    ident_h = np.eye(128, dtype=f32)
    identb_h = np.eye(128, dtype=f32).astype(bf)

    in_maps = []
    for m in range(n_cores):
        bs = slice(B * m, B * (m + 1))
        feat_h = np.ascontiguousarray(
            np.asarray(features[bs], f32).transpose(0, 2, 3, 1).reshape(B * HW, C)
        ).astype(bf)
        kp_h = np.ascontiguousarray(
            np.asarray(keypoint_coords[bs], f32).transpose(1, 0, 2).reshape(J, 2 * B)
        )
        in_maps.append({
            "feat": feat_h, "kp": kp_h,
            "w1o": w1o_h, "w1a": w1a_h, "w2o": w2o_h, "w2a": w2a_h,
            "b1o": b1o_h, "b1a": b1a_h, "b2o": b2o_h, "b2a": b2a_h,
            "bbase": bbase_h, "posc": posc_h,
            "ident": ident_h, "identb": identb_h,
        })
    return in_maps


_NC_CACHE = None


def get_nc():
    global _NC_CACHE
    if _NC_CACHE is None:
        _NC_CACHE = build_nc()
    return _NC_CACHE


def kernel(**inputs):
    from concourse.bass_utils import run_bass_kernel_spmd

    n_cores = 8
    nc = get_nc()
    in_maps = prepare_in_maps(**inputs, n_cores=n_cores)
    res = run_bass_kernel_spmd(
        nc, in_maps, core_ids=list(range(n_cores)),
        trace=bool(int(os.environ.get("KERNEL_TRACE", "0") or 0)),
    )
    kernel.last_results = res
    outs = [
        np.asarray(r["out"]).astype(np.float32).reshape(B, J, C)
        for r in res.results
    ]
    return np.concatenate(outs, axis=0)


# revision 14
# speedup vs baseline: 1.2893x; 1.0960x over previous
"""Trainium2 Bass kernel for nn_AdaptiveSampler (sparse grid_sample attention).

Strategy (data-parallel over batch, 8 cores x 4 batch items each):
  - Host: features reshaped channels-last [B*H*W, C] in bf16 so every
    spatial cell is one contiguous 2KB row -> indirect row gathers.
  - Device per core:
      phase A: keypoint -> bilinear corner cells/weights (DVE f32 math)
      seed    = dma_gather(4 corners x 512 keypoints) -> weighted reduce
      MLPs    = PE matmuls (offsets + attention logits), softmax on DVE/ACT
      phase B: per keypoint a 4x4 patch around the seed cell covers all
               16 sample corners; per-cell weights are built by position
               selects (d = floor(px) - patch_base), folding attention
               softmax + bilinear + border validity into one weight.
      fused   = dma_gather of 8KB patch rows (4 x-cells, 4 rows/keypoint)
                * broadcast weights, segment-reduce, PE-transpose, DMA out.
All computation (gathers, MLPs, softmax, bilinear) happens on-device; the
host only reorders input layout and concatenates per-core outputs.
"""

import os
import sys
from contextlib import ExitStack

import numpy as np

sys.path.insert(0, "/opt/trn_rl_repo")

import ml_dtypes

import concourse.bass as bass
import concourse.tile as tile
from concourse import bacc, mybir

F32 = mybir.dt.float32
BF16 = mybir.dt.bfloat16
I16 = mybir.dt.int16

ALU = mybir.AluOpType
ACT = mybir.ActivationFunctionType
AX = mybir.AxisListType

B = 4          # batch items per core
C = 1024       # channels
H = W = 64
HW = H * W     # 4096 cells per batch item
J = 128        # keypoints
NP = 4         # sample points per keypoint
Q = C // 128   # 8 channel chunks
NIDX = J * 16  # 2048 indices per gather set (seed corners / main patch rows)
TWO23 = float(2 ** 23)


def _floor(nc, pool, src, shape, tag):
    """floor(src) on DVE via round-to-nearest + correction. Returns tile."""
    rnd = pool.tile(list(shape), F32, tag=f"floor_rnd_{tag}")
    nc.vector.tensor_scalar(rnd[:], src, TWO23, TWO23, ALU.add, ALU.subtract)
    flo = pool.tile(list(shape), F32, tag=f"floor_out_{tag}")
    nc.vector.tensor_tensor(flo[:], src, rnd[:], ALU.is_lt)
    nc.vector.tensor_tensor(flo[:], rnd[:], flo[:], ALU.subtract)
    return flo


def build_nc():
    nc = bacc.Bacc()

    feat = nc.declare_dram_parameter("feat", [B * HW, C], BF16, isOutput=False)
    kp = nc.declare_dram_parameter("kp", [J, 2 * B], F32, isOutput=False)
    w1o = nc.declare_dram_parameter("w1o", [128, Q, 128], BF16, isOutput=False)
    w1a = nc.declare_dram_parameter("w1a", [128, Q, 128], BF16, isOutput=False)
    w2o = nc.declare_dram_parameter("w2o", [128, 8], BF16, isOutput=False)
    w2a = nc.declare_dram_parameter("w2a", [128, 4], BF16, isOutput=False)
    b1o = nc.declare_dram_parameter("b1o", [128, 1], F32, isOutput=False)
    b1a = nc.declare_dram_parameter("b1a", [128, 1], F32, isOutput=False)
    b2o = nc.declare_dram_parameter("b2o", [8, 1], F32, isOutput=False)
    b2a = nc.declare_dram_parameter("b2a", [4, 1], F32, isOutput=False)
    bbase = nc.declare_dram_parameter("bbase", [128, B], F32, isOutput=False)
    posc = nc.declare_dram_parameter("posc", [128, 4], F32, isOutput=False)
    ident = nc.declare_dram_parameter("ident", [128, 128], F32, isOutput=False)
    identb = nc.declare_dram_parameter("identb", [128, 128], BF16, isOutput=False)
    out = nc.declare_dram_parameter("out", [B * J, C], BF16, isOutput=True)

    # DRAM scratch for flattening per-column weights before partition bcast
    wscr_s = nc.dram_tensor("wscr_s", [J * 16], BF16)
    wscr_m = nc.dram_tensor("wscr_m", [J * 48], BF16)

    # Overlapping row view of feat: row i = cells i..i+3 (8KB), for patch
    # gathers. Max row start 16380 -> read end == tensor end exactly.
    feat_ov = bass.AP(feat[:].tensor, 0, [[C, B * HW - 2], [1, 3 * C]])

    with ExitStack() as ctx:
        tc = ctx.enter_context(tile.TileContext(nc))
        cons = ctx.enter_context(tc.tile_pool(name="cons", bufs=1))
        a = ctx.enter_context(tc.tile_pool(name="phaseA", bufs=1))
        gp = ctx.enter_context(tc.tile_pool(name="gather", bufs=3))
        wp = ctx.enter_context(tc.tile_pool(name="wbc", bufs=1))
        op = ctx.enter_context(tc.tile_pool(name="outT", bufs=2))
        ip = ctx.enter_context(tc.tile_pool(name="idxw", bufs=2))
        ps = ctx.enter_context(tc.tile_pool(name="psT", bufs=3, space="PSUM"))
        pmm = ctx.enter_context(tc.tile_pool(name="psMM", bufs=2, space="PSUM"))

        # ---------------- constants ----------------
        def c_load(name, shape, dt, src):
            t = cons.tile(shape, dt, tag=name)
            nc.sync.dma_start(out=t[:], in_=src)
            return t

        kp_sb = c_load("kp", [J, B, 2], F32, kp[:].rearrange("j (b t) -> j b t", t=2))
        w1o_sb = c_load("w1o", [128, Q, 128], BF16, w1o[:])
        w1a_sb = c_load("w1a", [128, Q, 128], BF16, w1a[:])
        w2o_sb = c_load("w2o", [128, 8], BF16, w2o[:])
        w2a_sb = c_load("w2a", [128, 4], BF16, w2a[:])
        b1o_sb = c_load("b1o", [128, 1], F32, b1o[:])
        b1a_sb = c_load("b1a", [128, 1], F32, b1a[:])
        b2o_sb = c_load("b2o", [8, 1], F32, b2o[:])
        b2a_sb = c_load("b2a", [4, 1], F32, b2a[:])
        bbase_sb = c_load("bbase", [128, B], F32, bbase[:])
        posc_sb = c_load("posc", [128, 4], F32, posc[:])
        id_sb = c_load("ident", [128, 128], F32, ident[:])
        idb_sb = c_load("identb", [128, 128], BF16, identb[:])

        # ---------------- phase A: seed corners ----------------
        ix = a.tile([J, B], F32)
        nc.vector.tensor_scalar(ix[:], kp_sb[:, :, 0], 31.5, 31.5, ALU.mult, ALU.add)
        iy = a.tile([J, B], F32)
        nc.vector.tensor_scalar(iy[:], kp_sb[:, :, 1], 31.5, 31.5, ALU.mult, ALU.add)

        x0 = _floor(nc, a, ix[:], (J, B), "x0")
        y0 = _floor(nc, a, iy[:], (J, B), "y0")

        def pair_and_weights(base, i_coord, tagp):
            p = a.tile([J, B, 2], F32, tag=f"{tagp}_p")
            wgt = a.tile([J, B, 2], F32, tag=f"{tagp}_w")
            nc.vector.tensor_copy(p[:, :, 0], base[:])
            nc.vector.tensor_scalar_add(p[:, :, 1], base[:], 1.0)
            nc.vector.tensor_tensor(wgt[:, :, 1], i_coord, base[:], ALU.subtract)
            nc.vector.tensor_scalar(
                wgt[:, :, 0], wgt[:, :, 1], -1.0, 1.0, ALU.mult, ALU.add
            )
            return p, wgt

        xp, wxp = pair_and_weights(x0, ix[:], "x")
        yp, wyp = pair_and_weights(y0, iy[:], "y")

        # seed cell idx [J, B, 2cy, 2cx] = bbase + yp*64 + xp
        idx4 = a.tile([J, B, 2, 2], F32)
        t1 = a.tile([J, B, 2], F32)
        nc.vector.tensor_scalar_mul(t1[:], yp[:], 64.0)
        nc.vector.tensor_tensor(
            idx4[:],
            t1[:].unsqueeze(3).to_broadcast((J, B, 2, 2)),
            xp[:].unsqueeze(2).to_broadcast((J, B, 2, 2)),
            ALU.add,
        )
        nc.vector.tensor_tensor(
            idx4[:],
            idx4[:],
            bbase_sb[:].unsqueeze(2).unsqueeze(3).to_broadcast((J, B, 2, 2)),
            ALU.add,
        )
        w4 = a.tile([J, B, 2, 2], F32)
        nc.vector.tensor_tensor(
            w4[:],
            wyp[:].unsqueeze(3).to_broadcast((J, B, 2, 2)),
            wxp[:].unsqueeze(2).to_broadcast((J, B, 2, 2)),
            ALU.mult,
        )

        def wrap_idx(idx_flat_ap):
            """[J,16] f32 cell ids -> wrapped+replicated [128, J] int16 tile."""
            rep = ip.tile([J, 8, 16], F32, tag="idxrep")
            for g in range(8):
                nc.vector.tensor_copy(rep[:, g, :], idx_flat_ap)
            psT = ps.tile([128, J], F32, tag="tp")
            nc.tensor.transpose(
                psT[:], rep[:].rearrange("j g c -> j (g c)"), id_sb[:, :J]
            )
            idxw = ip.tile([128, J], I16, tag="idxw")
            nc.vector.tensor_copy(idxw[:], psT[:])
            return idxw

        def bcast_weights(w_flat_ap, wscr, n, slot, dest_view=None):
            """[J, n] f32 col-weights -> [128, J*n] bf16 via DRAM bounce.
            dest_view(wscr_ap) may reorder the DRAM layout."""
            wb16 = a.tile([J, n], BF16, tag=f"wb16_{slot}")
            nc.vector.tensor_copy(wb16[:], w_flat_ap)
            dst = (
                dest_view(wscr[:])
                if dest_view is not None
                else wscr[:].rearrange("(j c) -> j c", c=n)
            )
            nc.sync.dma_start(out=dst, in_=wb16[:])
            wbc = wp.tile([128, J * n], BF16, tag=f"wbc_{slot}")
            nc.sync.dma_start(
                out=wbc[:],
                in_=wscr[:].unsqueeze(0).to_broadcast((128, J * n)),
            )
            return wbc

        idxw_seed = wrap_idx(idx4[:].rearrange("j b cy cx -> j (b cy cx)"))
        wbc_seed = bcast_weights(
            w4[:].rearrange("j b cy cx -> j (b cy cx)"), wscr_s, 16, "s"
        )

        # ---------------- seed gather + combine ----------------
        HN = 256  # seed chunk: 256 idx x 2KB rows (130 descs -> pipelined)
        seed = a.tile([128, Q, J * B], BF16)
        for h in range(8):
            seedg = gp.tile([128, Q, HN], BF16, tag="seedg")
            nc.gpsimd.dma_gather(
                seedg[:],
                feat[:],
                idxw_seed[:, 16 * h : 16 * h + 16],
                num_idxs=HN,
                num_idxs_reg=HN,
                elem_size=C,
                transpose=True,
            )
            with nc.allow_low_precision("bf16 grid-sample compute"):
                nc.vector.tensor_tensor(
                    seedg[:],
                    seedg[:],
                    wbc_seed[:, HN * h : HN * (h + 1)]
                    .unsqueeze(1)
                    .to_broadcast((128, Q, HN)),
                    ALU.mult,
                )
                nc.vector.tensor_reduce(
                    seed[:, :, 64 * h : 64 * (h + 1)],
                    seedg[:].rearrange("p q (jb c) -> p (q jb) c", c=4),
                    AX.X,
                    ALU.add,
                )

        # ---------------- MLPs ----------------
        def mlp_head(w1_sb, b1_sb, name):
            hps = pmm.tile([128, J * B], F32, tag="mm")
            for q in range(Q):
                nc.tensor.matmul(
                    hps[:],
                    w1_sb[:, q, :],
                    seed[:, q, :],
                    start=(q == 0),
                    stop=(q == Q - 1),
                )
            h_sb = a.tile([128, J * B], BF16, tag=f"hsb_{name}")
            nc.scalar.activation(h_sb[:], hps[:], ACT.Relu, bias=b1_sb[:])
            return h_sb

        h_off = mlp_head(w1o_sb, b1o_sb, "off")
        h_att = mlp_head(w1a_sb, b1a_sb, "att")

        ops2 = pmm.tile([8, J * B], F32, tag="mm")
        nc.tensor.matmul(ops2[:], w2o_sb[:], h_off[:], start=True, stop=True)
        off2 = a.tile([8, J * B], F32)
        nc.scalar.activation(off2[:], ops2[:], ACT.Identity, bias=b2o_sb[:])

        aps2 = pmm.tile([4, J * B], F32, tag="mm")
        nc.tensor.matmul(aps2[:], w2a_sb[:], h_att[:], start=True, stop=True)
        att2 = a.tile([4, J * B], F32)
        nc.scalar.activation(att2[:], aps2[:], ACT.Identity, bias=b2a_sb[:])

        # transpose MLP outputs back to [J, B, ch] layout (per-b strided cols)
        offT = a.tile([J, B, 8], F32)
        attT = a.tile([J, B, 4], F32)
        for b in range(B):
            pso = ps.tile([J, 8], F32, tag="tp")
            nc.tensor.transpose(pso[:], off2[:, b::B], id_sb[:8, :8])
            nc.scalar.copy(offT[:, b, :], pso[:])
            psa = ps.tile([J, 4], F32, tag="tp")
            nc.tensor.transpose(psa[:], att2[:, b::B], id_sb[:4, :4])
            nc.scalar.copy(attT[:, b, :], psa[:])

        # ---------------- phase B: 4x4 patch per keypoint ----------------
        # patch base bx/by [J, B] = clip(seed_corner - 1, 0, 60)
        rx = a.tile([J, B], F32)
        nc.vector.tensor_scalar(rx[:], ix[:], TWO23, TWO23, ALU.add, ALU.subtract)
        bx = a.tile([J, B], F32)
        nc.vector.tensor_scalar(bx[:], rx[:], -1.0, 0.0, ALU.add, ALU.max)
        nc.vector.tensor_scalar_min(bx[:], bx[:], 61.0)
        by = a.tile([J, B], F32)
        nc.vector.tensor_scalar(by[:], y0[:], -1.0, 0.0, ALU.add, ALU.max)
        nc.vector.tensor_scalar_min(by[:], by[:], 60.0)

        # per-point coords px/py [J, B, NP]
        px = a.tile([J, B, NP], F32)
        nc.vector.tensor_tensor(
            px[:],
            ix[:].unsqueeze(2).to_broadcast((J, B, NP)),
            offT[:, :, 0:NP],
            ALU.add,
        )
        py = a.tile([J, B, NP], F32)
        nc.vector.tensor_tensor(
            py[:],
            iy[:].unsqueeze(2).to_broadcast((J, B, NP)),
            offT[:, :, NP : 2 * NP],
            ALU.add,
        )

        # softmax over NP  [J, B, NP]
        amax = a.tile([J, B, 1], F32)
        nc.vector.tensor_reduce(amax[:], attT[:], AX.X, ALU.max)
        ae = a.tile([J, B, NP], F32)
        nc.vector.tensor_tensor(
            ae[:], attT[:], amax[:].to_broadcast((J, B, NP)), ALU.subtract
        )
        nc.scalar.activation(ae[:], ae[:], ACT.Exp)
        asum = a.tile([J, B, 1], F32)
        nc.vector.tensor_reduce(asum[:], ae[:], AX.X, ALU.add)
        nc.vector.reciprocal(asum[:], asum[:])
        attw = a.tile([J, B, NP], F32)
        nc.vector.tensor_tensor(
            attw[:], ae[:], asum[:].to_broadcast((J, B, NP)), ALU.mult
        )

        def axis_select(pc, base, tagp, npos=4):
            """Position-select weights [J, B, NP, npos]:
            w0*(pos==d) + w1*(pos==d+1), d = floor(pc) - base."""
            c0 = _floor(nc, a, pc[:], (J, B, NP), tagp)
            w1t = a.tile([J, B, NP], F32, tag=f"{tagp}_w1")
            nc.vector.tensor_tensor(w1t[:], pc[:], c0[:], ALU.subtract)
            w0t = a.tile([J, B, NP], F32, tag=f"{tagp}_w0")
            nc.vector.tensor_scalar(w0t[:], w1t[:], -1.0, 1.0, ALU.mult, ALU.add)
            d = a.tile([J, B, NP], F32, tag=f"{tagp}_d")
            nc.vector.tensor_tensor(
                d[:], c0[:], base[:].unsqueeze(2).to_broadcast((J, B, NP)),
                ALU.subtract,
            )
            d1 = a.tile([J, B, NP], F32, tag=f"{tagp}_d1")
            nc.vector.tensor_scalar_add(d1[:], d[:], 1.0)
            posb = (
                posc_sb[:, 0:npos]
                .unsqueeze(1)
                .unsqueeze(2)
                .to_broadcast((J, B, NP, npos))
            )
            sel = a.tile([J, B, NP, npos], F32, tag=f"{tagp}_sel")
            eq = a.tile([J, B, NP, npos], F32, tag=f"{tagp}_eq")
            nc.vector.tensor_tensor(
                eq[:], d[:].unsqueeze(3).to_broadcast((J, B, NP, npos)), posb,
                ALU.is_equal,
            )
            nc.vector.tensor_tensor(
                sel[:], eq[:], w0t[:].unsqueeze(3).to_broadcast((J, B, NP, npos)),
                ALU.mult,
            )
            nc.vector.tensor_tensor(
                eq[:], d1[:].unsqueeze(3).to_broadcast((J, B, NP, npos)), posb,
                ALU.is_equal,
            )
            nc.vector.tensor_tensor(
                eq[:], eq[:], w1t[:].unsqueeze(3).to_broadcast((J, B, NP, npos)),
                ALU.mult,
            )
            nc.vector.tensor_tensor(sel[:], sel[:], eq[:], ALU.add)
            return sel

        wxsel = axis_select(px, bx, "sx", npos=3)
        wysel = axis_select(py, by, "sy")

        # fold attention weight into y-selects: ty [J, B, NP, 4Y]
        ty = a.tile([J, B, NP, 4], F32)
        nc.vector.tensor_tensor(
            ty[:], wysel[:], attw[:].unsqueeze(3).to_broadcast((J, B, NP, 4)),
            ALU.mult,
        )
        # patch weights w43 [J, B, 4Y, 3X] = sum_n ty[n, Y] * wxsel[n, X]
        w43 = a.tile([J, B, 4, 3], F32)
        tmp43 = a.tile([J, B, 4, 3], F32)
        for n in range(NP):
            dst = w43 if n == 0 else tmp43
            nc.vector.tensor_tensor(
                dst[:],
                ty[:, :, n, :].unsqueeze(3).to_broadcast((J, B, 4, 3)),
                wxsel[:, :, n, :].unsqueeze(2).to_broadcast((J, B, 4, 3)),
                ALU.mult,
            )
            if n > 0:
                nc.vector.tensor_tensor(w43[:], w43[:], tmp43[:], ALU.add)

        # patch row ids [J, B, 4Y] = bbase + (by + Y)*64 + bx
        pbase = a.tile([J, B], F32)
        nc.vector.tensor_scalar_mul(pbase[:], by[:], 64.0)
        nc.vector.tensor_tensor(pbase[:], pbase[:], bx[:], ALU.add)
        nc.vector.tensor_tensor(pbase[:], pbase[:], bbase_sb[:], ALU.add)
        y64 = a.tile([128, 4], F32)
        nc.vector.tensor_scalar_mul(y64[:], posc_sb[:], 64.0)
        idxp = a.tile([J, B, 4], F32)
        nc.vector.tensor_tensor(
            idxp[:],
            pbase[:].unsqueeze(2).to_broadcast((J, B, 4)),
            y64[:].unsqueeze(1).to_broadcast((J, B, 4)),
            ALU.add,
        )

        idxw_m = wrap_idx(idxp[:].rearrange("j b y -> j (b y)"))
        wbc_m = bcast_weights(
            w43[:].rearrange("j b y x -> j (b y x)"), wscr_m, 48, "m",
            dest_view=lambda ap: ap.rearrange(
                "(x j b y) -> j (b y) x", x=3, j=J, b=B
            ),
        )

        # ---------------- main gather + fuse (16 chunks of 8 keypoints) ----
        MN = 128  # idxs per chunk: 8 keypoints x (4b x 4Y) 8KB patch rows
        fused = a.tile([128, Q, J * B], BF16)
        with nc.allow_low_precision("bf16 grid-sample compute"):
            for h in range(16):
                mg = gp.tile([128, 3 * Q, MN], BF16, tag="mg")
                nc.gpsimd.dma_gather(
                    mg[:],
                    feat_ov,
                    idxw_m[:, 8 * h : 8 * h + 8],
                    num_idxs=MN,
                    num_idxs_reg=MN,
                    elem_size=3 * C,
                    elem_step=C,
                    transpose=True,
                )
                mv = mg[:].rearrange("p (x q) i -> p x q i", x=3)
                for x in range(3):
                    nc.vector.tensor_tensor(
                        mv[:, x, :, :],
                        mv[:, x, :, :],
                        wbc_m[:, 2048 * x + 128 * h : 2048 * x + 128 * h + 128]
                        .unsqueeze(1)
                        .to_broadcast((128, Q, MN)),
                        ALU.mult,
                    )
                # sum over x: two adds on contiguous views
                nc.vector.tensor_tensor(
                    mg[:, 0:8, :], mg[:, 0:8, :], mg[:, 8:16, :], ALU.add
                )
                nc.vector.tensor_tensor(
                    mg[:, 0:8, :], mg[:, 0:8, :], mg[:, 16:24, :], ALU.add
                )
                fsl = fused[:, :, 32 * h : 32 * h + 32]
                nc.vector.tensor_reduce(
                    fsl,
                    mg[:, 0:8, :].rearrange("p q (jb y) -> p (q jb) y", y=4),
                    AX.X,
                    ALU.add,
                )

        # ---------------- output transpose + store ----------------
        for b in range(B):
            outT = op.tile([J, Q, 128], BF16, tag="outT")
            for q in range(Q):
                pst = ps.tile([J, 128], BF16, tag="tpb")
                nc.tensor.transpose(pst[:], fused[:, q, b::B], idb_sb[:, :J])
                nc.scalar.copy(outT[:, q, :], pst[:])
            nc.sync.dma_start(
                out=out[b * J : (b + 1) * J, :].rearrange(
                    "j (q c) -> j q c", q=Q
                ),
                in_=outT[:],
            )

    nc.finalize()
    return nc


def prepare_in_maps(features, keypoint_coords, w_off1, b_off1, w_off2, b_off2,
                    w_att1, b_att1, w_att2, b_att2, n_cores=8):
    bf = ml_dtypes.bfloat16
    f32 = np.float32

    def w1t(w):  # [128, C] -> [128 k_local, Q, 128 m] bf16
        return np.ascontiguousarray(
            w.T.reshape(Q, 128, 128).transpose(1, 0, 2).astype(bf)
        )

    w1o_h = w1t(np.asarray(w_off1, f32))
    w1a_h = w1t(np.asarray(w_att1, f32))
    w2o_h = np.ascontiguousarray(
        np.concatenate([w_off2[0::2], w_off2[1::2]], 0).T.astype(bf)
    )
    w2a_h = np.ascontiguousarray(np.asarray(w_att2, f32).T.astype(bf))
    b1o_h = np.asarray(b_off1, f32).reshape(128, 1).copy()
    b1a_h = np.asarray(b_att1, f32).reshape(128, 1).copy()
    b2o_h = np.concatenate([b_off2[0::2], b_off2[1::2]]).astype(f32).reshape(8, 1)
    b2a_h = np.asarray(b_att2, f32).reshape(4, 1).copy()
    bbase_h = np.broadcast_to(
        (np.arange(B, dtype=f32) * HW)[None, :], (128, B)
    ).copy()
    posc_h = np.broadcast_to(np.arange(4, dtype=f32)[None, :], (128, 4)).copy()
    ident_h = np.eye(128, dtype=f32)
    identb_h = np.eye(128, dtype=f32).astype(bf)

    in_maps = []
    for m in range(n_cores):
        bs = slice(B * m, B * (m + 1))
        feat_h = np.ascontiguousarray(
            np.asarray(features[bs], f32).transpose(0, 2, 3, 1).reshape(B * HW, C)
        ).astype(bf)
        kp_h = np.ascontiguousarray(
            np.asarray(keypoint_coords[bs], f32).transpose(1, 0, 2).reshape(J, 2 * B)
        )
        in_maps.append({
            "feat": feat_h, "kp": kp_h,
            "w1o": w1o_h, "w1a": w1a_h, "w2o": w2o_h, "w2a": w2a_h,
            "b1o": b1o_h, "b1a": b1a_h, "b2o": b2o_h, "b2a": b2a_h,
            "bbase": bbase_h, "posc": posc_h,
            "ident": ident_h, "identb": identb_h,
        })
    return in_maps


_NC_CACHE = None


def get_nc():
    global _NC_CACHE
    if _NC_CACHE is None:
        _NC_CACHE = build_nc()
    return _NC_CACHE


def kernel(**inputs):
    from concourse.bass_utils import run_bass_kernel_spmd

    n_cores = 8
    nc = get_nc()
    in_maps = prepare_in_maps(**inputs, n_cores=n_cores)
    res = run_bass_kernel_spmd(
        nc, in_maps, core_ids=list(range(n_cores)),
        trace=bool(int(os.environ.get("KERNEL_TRACE", "0") or 0)),
    )
    kernel.last_results = res
    outs = [
        np.asarray(r["out"]).astype(np.float32).reshape(B, J, C)
        for r in res.results
    ]
    return np.concatenate(outs, axis=0)


# revision 15
# speedup vs baseline: 1.6916x; 1.3120x over previous
"""Trainium2 Bass kernel for nn_AdaptiveSampler (sparse grid_sample attention).

Strategy (data-parallel over batch, 8 cores x 4 batch items each):
  - Host: features reshaped channels-last [B*H*W, C] in bf16 so every
    spatial cell is one contiguous 2KB row -> indirect row gathers.
  - Device per core:
      phase A: keypoint -> bilinear corner cells/weights (DVE f32 math)
      seed    = dma_gather(4 corners x 512 keypoints) -> weighted reduce
      MLPs    = PE matmuls (offsets + attention logits), softmax on DVE/ACT
      phase B: per keypoint a 4x4 patch around the seed cell covers all
               16 sample corners; per-cell weights are built by position
               selects (d = floor(px) - patch_base), folding attention
               softmax + bilinear + border validity into one weight.
      fused   = dma_gather of 8KB patch rows (4 x-cells, 4 rows/keypoint)
                * broadcast weights, segment-reduce, PE-transpose, DMA out.
All computation (gathers, MLPs, softmax, bilinear) happens on-device; the
host only reorders input layout and concatenates per-core outputs.
"""

import os
import sys
from contextlib import ExitStack

import numpy as np

sys.path.insert(0, "/opt/trn_rl_repo")

import ml_dtypes

import concourse.bass as bass
import concourse.tile as tile
from concourse import bacc, mybir

F32 = mybir.dt.float32
BF16 = mybir.dt.bfloat16
I16 = mybir.dt.int16

ALU = mybir.AluOpType
ACT = mybir.ActivationFunctionType
AX = mybir.AxisListType

B = 4          # batch items per core
C = 1024       # channels
H = W = 64
HW = H * W     # 4096 cells per batch item
J = 128        # keypoints
NP = 4         # sample points per keypoint
Q = C // 128   # 8 channel chunks
NIDX = J * 16  # 2048 indices per gather set (seed corners / main patch rows)
TWO23 = float(2 ** 23)


def _floor(nc, pool, src, shape, tag):
    """floor(src) on DVE via round-to-nearest + correction. Returns tile."""
    rnd = pool.tile(list(shape), F32, tag=f"floor_rnd_{tag}")
    nc.vector.tensor_scalar(rnd[:], src, TWO23, TWO23, ALU.add, ALU.subtract)
    flo = pool.tile(list(shape), F32, tag=f"floor_out_{tag}")
    nc.vector.tensor_tensor(flo[:], src, rnd[:], ALU.is_lt)
    nc.vector.tensor_tensor(flo[:], rnd[:], flo[:], ALU.subtract)
    return flo


def build_nc():
    nc = bacc.Bacc()

    feat = nc.declare_dram_parameter("feat", [B * HW, C], BF16, isOutput=False)
    kp = nc.declare_dram_parameter("kp", [J, 2 * B], F32, isOutput=False)
    w1o = nc.declare_dram_parameter("w1o", [128, Q, 128], BF16, isOutput=False)
    w1a = nc.declare_dram_parameter("w1a", [128, Q, 128], BF16, isOutput=False)
    w2o = nc.declare_dram_parameter("w2o", [128, 8], BF16, isOutput=False)
    w2a = nc.declare_dram_parameter("w2a", [128, 4], BF16, isOutput=False)
    b1o = nc.declare_dram_parameter("b1o", [128, 1], F32, isOutput=False)
    b1a = nc.declare_dram_parameter("b1a", [128, 1], F32, isOutput=False)
    b2o = nc.declare_dram_parameter("b2o", [8, 1], F32, isOutput=False)
    b2a = nc.declare_dram_parameter("b2a", [4, 1], F32, isOutput=False)
    bbase = nc.declare_dram_parameter("bbase", [128, B], F32, isOutput=False)
    posc = nc.declare_dram_parameter("posc", [128, 4], F32, isOutput=False)
    ident = nc.declare_dram_parameter("ident", [128, 128], F32, isOutput=False)
    identb = nc.declare_dram_parameter("identb", [128, 128], BF16, isOutput=False)
    out = nc.declare_dram_parameter("out", [B * J, C], BF16, isOutput=True)

    # DRAM scratch for flattening per-column weights before partition bcast
    wscr_s = nc.dram_tensor("wscr_s", [J * 16], BF16)
    wscr_m = nc.dram_tensor("wscr_m", [J * 48], BF16)

    # Overlapping row view of feat: row i = cells i..i+3 (8KB), for patch
    # gathers. Max row start 16380 -> read end == tensor end exactly.
    feat_ov = bass.AP(feat[:].tensor, 0, [[C, B * HW - 2], [1, 3 * C]])

    with ExitStack() as ctx:
        tc = ctx.enter_context(tile.TileContext(nc))
        cons = ctx.enter_context(tc.tile_pool(name="cons", bufs=1))
        a = ctx.enter_context(tc.tile_pool(name="phaseA", bufs=1))
        gp = ctx.enter_context(tc.tile_pool(name="gather", bufs=5))
        wp = ctx.enter_context(tc.tile_pool(name="wbc", bufs=1))
        op = ctx.enter_context(tc.tile_pool(name="outT", bufs=2))
        ip = ctx.enter_context(tc.tile_pool(name="idxw", bufs=2))
        ps = ctx.enter_context(tc.tile_pool(name="psT", bufs=3, space="PSUM"))
        pmm = ctx.enter_context(tc.tile_pool(name="psMM", bufs=2, space="PSUM"))

        # ---------------- constants ----------------
        def c_load(name, shape, dt, src):
            t = cons.tile(shape, dt, tag=name)
            nc.sync.dma_start(out=t[:], in_=src)
            return t

        kp_sb = c_load("kp", [J, B, 2], F32, kp[:].rearrange("j (b t) -> j b t", t=2))
        w1o_sb = c_load("w1o", [128, Q, 128], BF16, w1o[:])
        w1a_sb = c_load("w1a", [128, Q, 128], BF16, w1a[:])
        w2o_sb = c_load("w2o", [128, 8], BF16, w2o[:])
        w2a_sb = c_load("w2a", [128, 4], BF16, w2a[:])
        b1o_sb = c_load("b1o", [128, 1], F32, b1o[:])
        b1a_sb = c_load("b1a", [128, 1], F32, b1a[:])
        b2o_sb = c_load("b2o", [8, 1], F32, b2o[:])
        b2a_sb = c_load("b2a", [4, 1], F32, b2a[:])
        bbase_sb = c_load("bbase", [128, B], F32, bbase[:])
        posc_sb = c_load("posc", [128, 4], F32, posc[:])
        id_sb = c_load("ident", [128, 128], F32, ident[:])
        idb_sb = c_load("identb", [128, 128], BF16, identb[:])

        # ---------------- phase A: seed corners ----------------
        ix = a.tile([J, B], F32)
        nc.vector.tensor_scalar(ix[:], kp_sb[:, :, 0], 31.5, 31.5, ALU.mult, ALU.add)
        iy = a.tile([J, B], F32)
        nc.vector.tensor_scalar(iy[:], kp_sb[:, :, 1], 31.5, 31.5, ALU.mult, ALU.add)

        x0 = _floor(nc, a, ix[:], (J, B), "x0")
        y0 = _floor(nc, a, iy[:], (J, B), "y0")

        def pair_and_weights(base, i_coord, tagp):
            p = a.tile([J, B, 2], F32, tag=f"{tagp}_p")
            wgt = a.tile([J, B, 2], F32, tag=f"{tagp}_w")
            nc.vector.tensor_copy(p[:, :, 0], base[:])
            nc.vector.tensor_scalar_add(p[:, :, 1], base[:], 1.0)
            nc.vector.tensor_tensor(wgt[:, :, 1], i_coord, base[:], ALU.subtract)
            nc.vector.tensor_scalar(
                wgt[:, :, 0], wgt[:, :, 1], -1.0, 1.0, ALU.mult, ALU.add
            )
            return p, wgt

        xp, wxp = pair_and_weights(x0, ix[:], "x")
        yp, wyp = pair_and_weights(y0, iy[:], "y")

        # seed cell idx [J, B, 2cy, 2cx] = bbase + yp*64 + xp
        idx4 = a.tile([J, B, 2, 2], F32)
        t1 = a.tile([J, B, 2], F32)
        nc.vector.tensor_scalar_mul(t1[:], yp[:], 64.0)
        nc.vector.tensor_tensor(
            idx4[:],
            t1[:].unsqueeze(3).to_broadcast((J, B, 2, 2)),
            xp[:].unsqueeze(2).to_broadcast((J, B, 2, 2)),
            ALU.add,
        )
        nc.vector.tensor_tensor(
            idx4[:],
            idx4[:],
            bbase_sb[:].unsqueeze(2).unsqueeze(3).to_broadcast((J, B, 2, 2)),
            ALU.add,
        )
        w4 = a.tile([J, B, 2, 2], F32)
        nc.vector.tensor_tensor(
            w4[:],
            wyp[:].unsqueeze(3).to_broadcast((J, B, 2, 2)),
            wxp[:].unsqueeze(2).to_broadcast((J, B, 2, 2)),
            ALU.mult,
        )

        def wrap_idx(idx_flat_ap):
            """[J,16] f32 cell ids -> wrapped+replicated [128, J] int16 tile."""
            rep = ip.tile([J, 8, 16], F32, tag="idxrep")
            for g in range(8):
                nc.vector.tensor_copy(rep[:, g, :], idx_flat_ap)
            psT = ps.tile([128, J], F32, tag="tp")
            nc.tensor.transpose(
                psT[:], rep[:].rearrange("j g c -> j (g c)"), id_sb[:, :J]
            )
            idxw = ip.tile([128, J], I16, tag="idxw")
            nc.vector.tensor_copy(idxw[:], psT[:])
            return idxw

        def bcast_weights(w_flat_ap, wscr, n, slot, dest_view=None):
            """[J, n] f32 col-weights -> [128, J*n] bf16 via DRAM bounce.
            dest_view(wscr_ap) may reorder the DRAM layout."""
            wb16 = a.tile([J, n], BF16, tag=f"wb16_{slot}")
            nc.vector.tensor_copy(wb16[:], w_flat_ap)
            dst = (
                dest_view(wscr[:])
                if dest_view is not None
                else wscr[:].rearrange("(j c) -> j c", c=n)
            )
            nc.sync.dma_start(out=dst, in_=wb16[:])
            wbc = wp.tile([128, J * n], BF16, tag=f"wbc_{slot}")
            nc.sync.dma_start(
                out=wbc[:],
                in_=wscr[:].unsqueeze(0).to_broadcast((128, J * n)),
            )
            return wbc

        idxw_seed = wrap_idx(idx4[:].rearrange("j b cy cx -> j (b cy cx)"))
        wbc_seed = bcast_weights(
            w4[:].rearrange("j b cy cx -> j (b cy cx)"), wscr_s, 16, "s"
        )

        # ---------------- seed gather + combine ----------------
        HN = 256  # seed chunk: 256 idx x 2KB rows (130 descs -> pipelined)
        seed = a.tile([128, Q, J * B], BF16)
        for h in range(8):
            seedg = gp.tile([128, Q, HN], BF16, tag="seedg")
            nc.gpsimd.dma_gather(
                seedg[:],
                feat[:],
                idxw_seed[:, 16 * h : 16 * h + 16],
                num_idxs=HN,
                num_idxs_reg=HN,
                elem_size=C,
                transpose=True,
            )
            with nc.allow_low_precision("bf16 grid-sample compute"):
                nc.vector.tensor_tensor(
                    seedg[:],
                    seedg[:],
                    wbc_seed[:, HN * h : HN * (h + 1)]
                    .unsqueeze(1)
                    .to_broadcast((128, Q, HN)),
                    ALU.mult,
                )
                nc.vector.tensor_reduce(
                    seed[:, :, 64 * h : 64 * (h + 1)],
                    seedg[:].rearrange("p q (jb c) -> p (q jb) c", c=4),
                    AX.X,
                    ALU.add,
                )

        # ---------------- MLPs ----------------
        def mlp_head(w1_sb, b1_sb, name):
            hps = pmm.tile([128, J * B], F32, tag="mm")
            for q in range(Q):
                nc.tensor.matmul(
                    hps[:],
                    w1_sb[:, q, :],
                    seed[:, q, :],
                    start=(q == 0),
                    stop=(q == Q - 1),
                )
            h_sb = a.tile([128, J * B], BF16, tag=f"hsb_{name}")
            nc.scalar.activation(h_sb[:], hps[:], ACT.Relu, bias=b1_sb[:])
            return h_sb

        h_off = mlp_head(w1o_sb, b1o_sb, "off")
        h_att = mlp_head(w1a_sb, b1a_sb, "att")

        ops2 = pmm.tile([8, J * B], F32, tag="mm")
        nc.tensor.matmul(ops2[:], w2o_sb[:], h_off[:], start=True, stop=True)
        off2 = a.tile([8, J * B], F32)
        nc.scalar.activation(off2[:], ops2[:], ACT.Identity, bias=b2o_sb[:])

        aps2 = pmm.tile([4, J * B], F32, tag="mm")
        nc.tensor.matmul(aps2[:], w2a_sb[:], h_att[:], start=True, stop=True)
        att2 = a.tile([4, J * B], F32)
        nc.scalar.activation(att2[:], aps2[:], ACT.Identity, bias=b2a_sb[:])

        # transpose MLP outputs back to [J, B, ch] layout (per-b strided cols)
        offT = a.tile([J, B, 8], F32)
        attT = a.tile([J, B, 4], F32)
        for b in range(B):
            pso = ps.tile([J, 8], F32, tag="tp")
            nc.tensor.transpose(pso[:], off2[:, b::B], id_sb[:8, :8])
            nc.scalar.copy(offT[:, b, :], pso[:])
            psa = ps.tile([J, 4], F32, tag="tp")
            nc.tensor.transpose(psa[:], att2[:, b::B], id_sb[:4, :4])
            nc.scalar.copy(attT[:, b, :], psa[:])

        # ---------------- phase B: 4x4 patch per keypoint ----------------
        # patch base bx/by [J, B] = clip(seed_corner - 1, 0, 60)
        rx = a.tile([J, B], F32)
        nc.vector.tensor_scalar(rx[:], ix[:], TWO23, TWO23, ALU.add, ALU.subtract)
        bx = a.tile([J, B], F32)
        nc.vector.tensor_scalar(bx[:], rx[:], -1.0, 0.0, ALU.add, ALU.max)
        nc.vector.tensor_scalar_min(bx[:], bx[:], 61.0)
        by = a.tile([J, B], F32)
        nc.vector.tensor_scalar(by[:], y0[:], -1.0, 0.0, ALU.add, ALU.max)
        nc.vector.tensor_scalar_min(by[:], by[:], 60.0)

        # per-point coords px/py [J, B, NP]
        px = a.tile([J, B, NP], F32)
        nc.vector.tensor_tensor(
            px[:],
            ix[:].unsqueeze(2).to_broadcast((J, B, NP)),
            offT[:, :, 0:NP],
            ALU.add,
        )
        py = a.tile([J, B, NP], F32)
        nc.vector.tensor_tensor(
            py[:],
            iy[:].unsqueeze(2).to_broadcast((J, B, NP)),
            offT[:, :, NP : 2 * NP],
            ALU.add,
        )

        # softmax over NP  [J, B, NP]
        amax = a.tile([J, B, 1], F32)
        nc.vector.tensor_reduce(amax[:], attT[:], AX.X, ALU.max)
        ae = a.tile([J, B, NP], F32)
        nc.vector.tensor_tensor(
            ae[:], attT[:], amax[:].to_broadcast((J, B, NP)), ALU.subtract
        )
        nc.scalar.activation(ae[:], ae[:], ACT.Exp)
        asum = a.tile([J, B, 1], F32)
        nc.vector.tensor_reduce(asum[:], ae[:], AX.X, ALU.add)
        nc.vector.reciprocal(asum[:], asum[:])
        attw = a.tile([J, B, NP], F32)
        nc.vector.tensor_tensor(
            attw[:], ae[:], asum[:].to_broadcast((J, B, NP)), ALU.mult
        )

        def axis_select(pc, base, tagp, npos=4):
            """Position-select weights [J, B, NP, npos]:
            w0*(pos==d) + w1*(pos==d+1), d = floor(pc) - base."""
            c0 = _floor(nc, a, pc[:], (J, B, NP), tagp)
            w1t = a.tile([J, B, NP], F32, tag=f"{tagp}_w1")
            nc.vector.tensor_tensor(w1t[:], pc[:], c0[:], ALU.subtract)
            w0t = a.tile([J, B, NP], F32, tag=f"{tagp}_w0")
            nc.vector.tensor_scalar(w0t[:], w1t[:], -1.0, 1.0, ALU.mult, ALU.add)
            d = a.tile([J, B, NP], F32, tag=f"{tagp}_d")
            nc.vector.tensor_tensor(
                d[:], c0[:], base[:].unsqueeze(2).to_broadcast((J, B, NP)),
                ALU.subtract,
            )
            d1 = a.tile([J, B, NP], F32, tag=f"{tagp}_d1")
            nc.vector.tensor_scalar_add(d1[:], d[:], 1.0)
            posb = (
                posc_sb[:, 0:npos]
                .unsqueeze(1)
                .unsqueeze(2)
                .to_broadcast((J, B, NP, npos))
            )
            sel = a.tile([J, B, NP, npos], F32, tag=f"{tagp}_sel")
            eq = a.tile([J, B, NP, npos], F32, tag=f"{tagp}_eq")
            nc.vector.tensor_tensor(
                eq[:], d[:].unsqueeze(3).to_broadcast((J, B, NP, npos)), posb,
                ALU.is_equal,
            )
            nc.vector.tensor_tensor(
                sel[:], eq[:], w0t[:].unsqueeze(3).to_broadcast((J, B, NP, npos)),
                ALU.mult,
            )
            nc.vector.tensor_tensor(
                eq[:], d1[:].unsqueeze(3).to_broadcast((J, B, NP, npos)), posb,
                ALU.is_equal,
            )
            nc.vector.tensor_tensor(
                eq[:], eq[:], w1t[:].unsqueeze(3).to_broadcast((J, B, NP, npos)),
                ALU.mult,
            )
            nc.vector.tensor_tensor(sel[:], sel[:], eq[:], ALU.add)
            return sel

        wxsel = axis_select(px, bx, "sx", npos=3)
        wysel = axis_select(py, by, "sy")

        # fold attention weight into y-selects: ty [J, B, NP, 4Y]
        ty = a.tile([J, B, NP, 4], F32)
        nc.vector.tensor_tensor(
            ty[:], wysel[:], attw[:].unsqueeze(3).to_broadcast((J, B, NP, 4)),
            ALU.mult,
        )
        # patch weights w43 [J, 3X, B, 4Y] (x-outer for contiguous DRAM
        # bounce) = sum_n ty[n, Y] * wxsel[n, X]
        w43 = a.tile([J, 3, B, 4], F32)
        tmp43 = a.tile([J, 3, B, 4], F32)
        for n in range(NP):
            dst = (w43 if n == 0 else tmp43)[:].transpose([0, 2, 3, 1])
            nc.vector.tensor_tensor(
                dst,
                ty[:, :, n, :].unsqueeze(3).to_broadcast((J, B, 4, 3)),
                wxsel[:, :, n, :].unsqueeze(2).to_broadcast((J, B, 4, 3)),
                ALU.mult,
            )
            if n > 0:
                nc.vector.tensor_tensor(
                    w43[:].rearrange("j x b y -> j (x b y)"),
                    w43[:].rearrange("j x b y -> j (x b y)"),
                    tmp43[:].rearrange("j x b y -> j (x b y)"),
                    ALU.add,
                )

        # patch row ids [J, B, 4Y] = bbase + (by + Y)*64 + bx
        pbase = a.tile([J, B], F32)
        nc.vector.tensor_scalar_mul(pbase[:], by[:], 64.0)
        nc.vector.tensor_tensor(pbase[:], pbase[:], bx[:], ALU.add)
        nc.vector.tensor_tensor(pbase[:], pbase[:], bbase_sb[:], ALU.add)
        y64 = a.tile([128, 4], F32)
        nc.vector.tensor_scalar_mul(y64[:], posc_sb[:], 64.0)
        idxp = a.tile([J, B, 4], F32)
        nc.vector.tensor_tensor(
            idxp[:],
            pbase[:].unsqueeze(2).to_broadcast((J, B, 4)),
            y64[:].unsqueeze(1).to_broadcast((J, B, 4)),
            ALU.add,
        )

        idxw_m = wrap_idx(idxp[:].rearrange("j b y -> j (b y)"))
        wbc_m = bcast_weights(
            w43[:].rearrange("j x b y -> j (x b y)"), wscr_m, 48, "m",
            dest_view=lambda ap: ap.rearrange(
                "(x j c) -> j x c", x=3, j=J
            ),
        )

        # ---------------- main gather + fuse (16 chunks of 8 keypoints) ----
        MN = 128  # idxs per chunk: 8 keypoints x (4b x 4Y) 8KB patch rows
        fused = a.tile([128, Q, J * B], BF16)
        with nc.allow_low_precision("bf16 grid-sample compute"):
            for h in range(16):
                mg = gp.tile([128, 3 * Q, MN], BF16, tag="mg")
                nc.gpsimd.dma_gather(
                    mg[:],
                    feat_ov,
                    idxw_m[:, 8 * h : 8 * h + 8],
                    num_idxs=MN,
                    num_idxs_reg=MN,
                    elem_size=3 * C,
                    elem_step=C,
                    transpose=True,
                )
                mv = mg[:].rearrange("p (x q) i -> p x q i", x=3)
                for x in range(3):
                    nc.vector.tensor_tensor(
                        mv[:, x, :, :],
                        mv[:, x, :, :],
                        wbc_m[:, 2048 * x + 128 * h : 2048 * x + 128 * h + 128]
                        .unsqueeze(1)
                        .to_broadcast((128, Q, MN)),
                        ALU.mult,
                    )
                # sum over x: two adds on contiguous views
                nc.vector.tensor_tensor(
                    mg[:, 0:8, :], mg[:, 0:8, :], mg[:, 8:16, :], ALU.add
                )
                nc.vector.tensor_tensor(
                    mg[:, 0:8, :], mg[:, 0:8, :], mg[:, 16:24, :], ALU.add
                )
                fsl = fused[:, :, 32 * h : 32 * h + 32]
                nc.vector.tensor_reduce(
                    fsl,
                    mg[:, 0:8, :].rearrange("p q (jb y) -> p (q jb) y", y=4),
                    AX.X,
                    ALU.add,
                )

        # ---------------- output transpose + store ----------------
        for b in range(B):
            outT = op.tile([J, Q, 128], BF16, tag="outT")
            for q in range(Q):
                pst = ps.tile([J, 128], BF16, tag="tpb")
                nc.tensor.transpose(pst[:], fused[:, q, b::B], idb_sb[:, :J])
                nc.scalar.copy(outT[:, q, :], pst[:])
            nc.sync.dma_start(
                out=out[b * J : (b + 1) * J, :].rearrange(
                    "j (q c) -> j q c", q=Q
                ),
                in_=outT[:],
            )

    nc.finalize()
    return nc


def prepare_in_maps(features, keypoint_coords, w_off1, b_off1, w_off2, b_off2,
                    w_att1, b_att1, w_att2, b_att2, n_cores=8):
    bf = ml_dtypes.bfloat16
    f32 = np.float32

    def w1t(w):  # [128, C] -> [128 k_local, Q, 128 m] bf16
        return np.ascontiguousarray(
            w.T.reshape(Q, 128, 128).transpose(1, 0, 2).astype(bf)
        )

    w1o_h = w1t(np.asarray(w_off1, f32))
    w1a_h = w1t(np.asarray(w_att1, f32))
    w2o_h = np.ascontiguousarray(
        np.concatenate([w_off2[0::2], w_off2[1::2]], 0).T.astype(bf)
    )
    w2a_h = np.ascontiguousarray(np.asarray(w_att2, f32).T.astype(bf))
    b1o_h = np.asarray(b_off1, f32).reshape(128, 1).copy()
    b1a_h = np.asarray(b_att1, f32).reshape(128, 1).copy()
    b2o_h = np.concatenate([b_off2[0::2], b_off2[1::2]]).astype(f32).reshape(8, 1)
    b2a_h = np.asarray(b_att2, f32).reshape(4, 1).copy()
    bbase_h = np.broadcast_to(
        (np.arange(B, dtype=f32) * HW)[None, :], (128, B)
    ).copy()
    posc_h = np.broadcast_to(np.arange(4, dtype=f32)[None, :], (128, 4)).copy()
    ident_h = np.eye(128, dtype=f32)
    identb_h = np.eye(128, dtype=f32).astype(bf)

    in_maps = []
    for m in range(n_cores):
        bs = slice(B * m, B * (m + 1))
        feat_h = np.ascontiguousarray(
            np.asarray(features[bs], f32).transpose(0, 2, 3, 1).reshape(B * HW, C)
        ).astype(bf)
        kp_h = np.ascontiguousarray(
            np.asarray(keypoint_coords[bs], f32).transpose(1, 0, 2).reshape(J, 2 * B)
        )
        in_maps.append({
            "feat": feat_h, "kp": kp_h,
            "w1o": w1o_h, "w1a": w1a_h, "w2o": w2o_h, "w2a": w2a_h,
            "b1o": b1o_h, "b1a": b1a_h, "b2o": b2o_h, "b2a": b2a_h,
            "bbase": bbase_h, "posc": posc_h,
            "ident": ident_h, "identb": identb_h,
        })
    return in_maps


_NC_CACHE = None


def get_nc():
    global _NC_CACHE
    if _NC_CACHE is None:
        _NC_CACHE = build_nc()
    return _NC_CACHE


def kernel(**inputs):
    from concourse.bass_utils import run_bass_kernel_spmd

    n_cores = 8
    nc = get_nc()
    in_maps = prepare_in_maps(**inputs, n_cores=n_cores)
    res = run_bass_kernel_spmd(
        nc, in_maps, core_ids=list(range(n_cores)),
        trace=bool(int(os.environ.get("KERNEL_TRACE", "0") or 0)),
    )
    kernel.last_results = res
    outs = [
        np.asarray(r["out"]).astype(np.float32).reshape(B, J, C)
        for r in res.results
    ]
    return np.concatenate(outs, axis=0)
